# revision 36
# baseline (speedup 1.0000x reference)
"""Trainium2 Bass kernel for nn_MultiHeadAttention_63814624084186.

Reference computation (per batch sample b, fully independent across b):
  x: [512, 4096]  (C channels x N=64*64 pixels)
  qkv = w_qkv @ x            -> q,k,v each [512, 4096] (8 heads x 64 dims)
  scores = (q_h @ k_h^T)/8   -> [64, 64] per head   (channel-attention)
  attn = softmax(scores, -1)
  out_h = attn_h @ v_h       -> [64, 4096]
  y = w_out @ out + b_out    -> [512, 4096]
  y = groupnorm(y over all C,N) * gamma + beta

Key algebra (this version): attention is over the CHANNEL dim, so
  scores_h = q_h k_h^T = (w_q G w_k^T)_h   with  G = x x^T  [512,512]
  y = w_out bd(A) w_v x = W_eff x          with  W_eff folded on-chip
q, k, v are never materialized.  Per-batch PE work drops from ~4.5e9
MACs (qkv + v + out-proj) to ~2.4e9 (G + y GEMM + small folds).

Sharding: pure data-parallel over batch: 16 samples / 8 cores = 2 per core.

Pipeline (PE queue order; b0/b1 are the two per-core batches):
  G0 T0 sc0 | G1 | R0 W20 T1 sc1 | y0 | R1 W21 | y1 | tails
softmax(b) runs on DVE under the next long PE phase, so the PE never
waits on it.  GroupNorm: bn_stats per psum block, bias folded into the
cross-partition combine (ones-matmul), apply + writeout overlap y1.
"""

import numpy as np
from contextlib import ExitStack

import concourse.bass as bass
import concourse.tile as tile
from concourse import bacc, mybir
from concourse.bass_utils import run_bass_kernel_spmd
from concourse.masks import make_identity

F32 = mybir.dt.float32
F16 = mybir.dt.float16
AX = mybir.AxisListType
ALU = mybir.AluOpType
ACTF = mybir.ActivationFunctionType

B = 16          # global batch
C = 512         # channels
N = 4096        # pixels (64*64)
HW_SIDE = 64
NCORES = 8
PB = B // NCORES  # batches per core
P = 128
KC = C // P     # 4 channel chunks
NB = 8          # n blocks of 512 (y GEMM)
NT = 16         # xT tiles of 2 n-chunks each (G GEMM)
NS = N // 512   # 8 pixel chunks of 512
NHP = 4         # head pairs
XLOOK = 8       # xT DMA lookahead tiles
EPS = 1e-5


def build_nc():
    nc = bacc.Bacc("TRN2", target_bir_lowering=False, debug=False,
                   num_devices=NCORES)

    # xT[b, t, p, j*512+c] = x[b, c, (2t+j)*128 + p]
    xt_d = nc.declare_dram_parameter("xt", [PB, NT, P, 1024], F16, isOutput=False)
    # x[b, nb, p, k*512+n] = x[b, k*128+p, nb*512+n]
    x_d = nc.declare_dram_parameter("x", [PB, NB, P, KC * 512], F16, isOutput=False)
    wq_d = nc.declare_dram_parameter("wq", [P, KC, C], F16, isOutput=False)   # w_q^T
    wk_d = nc.declare_dram_parameter("wk", [P, KC, C], F16, isOutput=False)   # w_k^T
    wv_d = nc.declare_dram_parameter("wv", [P, KC, C], F16, isOutput=False)   # w_v
    wo_d = nc.declare_dram_parameter("wo", [P, KC, C], F16, isOutput=False)   # w_out^T
    bias_d = nc.declare_dram_parameter("bvec", [P, KC], F32, isOutput=False)
    gamma_d = nc.declare_dram_parameter("gamma", [P, KC], F32, isOutput=False)
    beta_d = nc.declare_dram_parameter("beta", [P, KC], F32, isOutput=False)
    out_d = nc.declare_dram_parameter("out", [PB, C, N], F16, isOutput=True)

    with tile.TileContext(nc) as tc, ExitStack() as ctx:
        consts = ctx.enter_context(tc.tile_pool(name="consts", bufs=1))
        xtpool = ctx.enter_context(tc.tile_pool(name="xtpool", bufs=XLOOK + 2))
        xpool = ctx.enter_context(tc.tile_pool(name="xpool", bufs=3))
        gpool = ctx.enter_context(tc.tile_pool(name="gpool", bufs=2))
        tpool = ctx.enter_context(tc.tile_pool(name="tpool", bufs=2))
        rpool = ctx.enter_context(tc.tile_pool(name="rpool", bufs=2))
        w2pool = ctx.enter_context(tc.tile_pool(name="w2pool", bufs=2))
        ypool = ctx.enter_context(tc.tile_pool(name="ypool", bufs=4))
        attn = ctx.enter_context(tc.tile_pool(name="attn", bufs=8))
        attnt = ctx.enter_context(tc.tile_pool(name="attnt", bufs=4))
        stats = ctx.enter_context(tc.tile_pool(name="stats", bufs=4))
        # psg serves both G (4 full banks) and the scores tiles: a matmul
        # start=True resets the target bank's whole per-partition row, so
        # each head-pair's score accumulator needs its own bank (partition
        # packing 0:64/64:128 within a bank is safe, free-offset packing
        # is NOT).  The pool rotation reuses G's banks once G is copied out.
        psg = ctx.enter_context(tc.tile_pool(name="psg", bufs=4, space="PSUM"))
        psmm = ctx.enter_context(tc.tile_pool(name="psmm", bufs=4, space="PSUM"))

        def load_w(dram):
            t = consts.tile([P, KC, C], F16, tag=f"w_{dram.name}")
            nc.sync.dma_start(out=t, in_=dram[:, :, :])
            return t

        xt_tiles = {}

        def fetch_xt(b, t):
            xt = xtpool.tile([P, 2, 512], F16, tag="xt", name=f"xt_{b}_{t}")
            nc.sync.dma_start(
                out=xt, in_=xt_d[b, t].rearrange("p (j c) -> p j c", j=2))
            xt_tiles[(b, t)] = xt

        # xT tiles first: weight loads are deferred until G0's xT stream is
        # fully issued (weights are only needed from T0 onward), so the
        # first G matmuls are never DMA-supply-gated.
        for t in range(XLOOK):
            fetch_xt(0, t)
        W = {}

        bias_sb = consts.tile([P, KC], F32, tag="bias")
        nc.gpsimd.dma_start(out=bias_sb, in_=bias_d[:, :])
        gamma_sb = consts.tile([P, KC], F32, tag="gamma")
        nc.gpsimd.dma_start(out=gamma_sb, in_=gamma_d[:, :])
        beta_sb = consts.tile([P, KC], F32, tag="beta")
        nc.gpsimd.dma_start(out=beta_sb, in_=beta_d[:, :])

        ident_sb = consts.tile([P, P], F16, tag="ident")
        make_identity(nc, ident_sb)
        eps_sb = consts.tile([1, 1], F32, tag="eps")
        nc.vector.memset(eps_sb, EPS)
        # pre-warm the exp activation table (softmax) so no ACT table load
        # lands mid-kernel; the stats chain's rsqrt runs DVE-only.
        warm_sb = consts.tile([1, 1], F32, tag="warm")
        nc.scalar.activation(out=warm_sb, in_=eps_sb, func=ACTF.Exp,
                             bias=0.0, scale=0.0)
        magic_sb = consts.tile([1, 1], mybir.dt.int32, tag="magic")
        nc.vector.memset(magic_sb, 0x5f3759df)
        c15_sb = consts.tile([1, 1], F32, tag="c15")
        nc.vector.memset(c15_sb, 1.5)
        # 1/C is folded into the reduction vector (saves a chain op)
        ones_col = consts.tile([P, 1], F32, tag="ones_col")
        nc.vector.memset(ones_col, 1.0 / C)
        ones_row = consts.tile([1, P], F32, tag="ones_row")
        nc.vector.memset(ones_row, 1.0)

        # per-batch state carried between emission stages
        st_g = {}    # G in SBUF (f16) [P, KC, C]
        st_t = {}    # T = G @ wk^T   [P, KC, C]
        st_sc = {}   # scores psum tiles (4x [P, 64], head-pair packed)
        st_at = {}   # block-diag attn tiles
        st_r = {}    # R = bd(A)^T @ wo^T
        st_w2 = {}   # W_effT = wv^T-contract @ R
        st_y = {}
        st_stats = {}
        st_scale = {}

        def emit_G(b, prefetched, hook=None):
            """G = x x^T, t-outer: each xT tile is consumed then retired.
            Only the upper block-triangle is computed (rhs = cols >= m*128);
            the 6 lower [128,128] blocks are PE-transposed from the upper
            copies.  All 4 chunk accumulators live in 4 psum banks."""
            g_sb = gpool.tile([P, KC, C], F16, tag="g", name=f"g_{b}")
            st_g[b] = g_sb
            ps = [psg.tile([P, C - m * P], F32, tag="psg", name=f"g_{b}_{m}")
                  for m in range(KC)]
            for t in range(NT):
                tf = t + prefetched
                if tf < NT:
                    fetch_xt(b, tf)
                elif b + 1 < PB and tf - NT < NT:
                    fetch_xt(b + 1, tf - NT)
                if hook and t in hook:
                    hook[t]()
                xt = xt_tiles.pop((b, t))
                for j in range(2):
                    for m in range(KC):
                        nc.tensor.matmul(
                            ps[m],
                            lhsT=xt[:, j, m * P:(m + 1) * P],
                            rhs=xt[:, j, m * P:],
                            start=(t == 0 and j == 0),
                            stop=(t == NT - 1 and j == 1),
                            skip_group_check=True)
            for m in range(KC):
                if m % 2 == 0:
                    nc.scalar.copy(out=g_sb[:, m, m * P:], in_=ps[m])
                else:
                    nc.vector.tensor_copy(out=g_sb[:, m, m * P:], in_=ps[m])
            # lower blocks (m, mp<m) = transpose(upper block (mp, m)),
            # ordered so T chunk 0's operands are ready first
            for m, mp in ((1, 0), (2, 0), (3, 0), (2, 1), (3, 1), (3, 2)):
                pst = psmm.tile([P, P], F16, tag="psmm")
                nc.tensor.transpose(
                    pst, g_sb[:, mp, m * P:(m + 1) * P], ident_sb)
                if (m + mp) % 2 == 0:
                    nc.vector.tensor_copy(
                        out=g_sb[:, m, mp * P:(mp + 1) * P], in_=pst)
                else:
                    nc.scalar.copy(
                        out=g_sb[:, m, mp * P:(mp + 1) * P], in_=pst)

        def emit_T(b):
            """T = G @ wk^T  [c, e], m-outer single-bank accumulation."""
            g_sb = st_g[b]
            t_sb = tpool.tile([P, KC, C], F16, tag="t", name=f"t_{b}")
            st_t[b] = t_sb
            for m in range(KC):
                ps = psmm.tile([P, C], F32, tag="psmm")
                for mp in range(KC):
                    nc.tensor.matmul(
                        ps,
                        lhsT=g_sb[:, mp, m * P:(m + 1) * P],
                        rhs=W['wk'][:, mp, :],
                        start=(mp == 0), stop=(mp == KC - 1))
                nc.vector.tensor_copy(out=t_sb[:, m, :], in_=ps)

        def emit_scores(b):
            """scores_h = (w_q T)_h, two heads packed per psum tile; k-outer
            so the first matmuls only need T chunk 0."""
            t_sb = st_t[b]
            sc_ps = [psg.tile([P, 64], F32, tag="psg", name=f"sc_{b}_{hp}")
                     for hp in range(NHP)]
            st_sc[b] = sc_ps
            for k in range(KC):
                for hp in range(NHP):
                    hA, hB = 2 * hp, 2 * hp + 1
                    clA = slice(hA * 64, hA * 64 + 64)
                    clB = slice(hB * 64, hB * 64 + 64)
                    nc.tensor.matmul(
                        sc_ps[hp][0:64, :],
                        lhsT=W['wq'][:, k, clA], rhs=t_sb[:, k, clA],
                        start=(k == 0), stop=(k == KC - 1),
                        skip_group_check=True)
                    nc.tensor.matmul(
                        sc_ps[hp][64:P, :],
                        lhsT=W['wq'][:, k, clB], rhs=t_sb[:, k, clB],
                        start=(k == 0), stop=(k == KC - 1),
                        skip_group_check=True)

        def emit_softmax(b):
            """softmax over scores (all head pairs batched) -> blockdiag tiles."""
            sc_ps = st_sc[b]
            a_all = attn.tile([P, NHP, 64], F32, tag="a_all")
            for hp in range(NHP):
                nc.vector.tensor_copy(out=a_all[:, hp, :], in_=sc_ps[hp])
            mx = attn.tile([P, NHP, 1], F32, tag="mx4")
            nc.vector.reduce_max(out=mx, in_=a_all, axis=AX.X)
            d_all = attn.tile([P, NHP, 64], F32, tag="d_all")
            nc.vector.tensor_tensor(d_all, a_all,
                                    mx.to_broadcast([P, NHP, 64]), ALU.subtract)
            e_all = attn.tile([P, NHP, 64], F32, tag="e_all")
            nc.scalar.activation(out=e_all, in_=d_all, func=ACTF.Exp,
                                 bias=0.0, scale=0.125)
            sm = attn.tile([P, NHP, 1], F32, tag="sm4")
            nc.vector.reduce_sum(out=sm, in_=e_all, axis=AX.X)
            rs = attn.tile([P, NHP, 1], F32, tag="rs4")
            nc.vector.reciprocal(out=rs, in_=sm)
            a_mm = attn.tile([P, NHP, 64], F16, tag="amm4")
            nc.vector.tensor_tensor(a_mm, e_all,
                                    rs.to_broadcast([P, NHP, 64]), ALU.mult)
            bd_tiles = []
            for hp in range(NHP):
                at = attnt.tile([P, P], F16, tag="attnT", name=f"at_{b}_{hp}")
                nc.gpsimd.memset(at, 0.0)
                nc.vector.tensor_copy(out=at[0:64, 0:64], in_=a_mm[0:64, hp, :])
                nc.vector.tensor_copy(out=at[64:P, 64:P], in_=a_mm[64:P, hp, :])
                bd_tiles.append(at)
            st_at[b] = bd_tiles

        def emit_R(b):
            """R[e, o] = sum_d bd(A)[d, e] wo^T[d, o]."""
            bd_tiles = st_at[b]
            r_sb = rpool.tile([P, KC, C], F16, tag="r", name=f"r_{b}")
            st_r[b] = r_sb
            for hp in range(NHP):
                ps = psmm.tile([P, C], F32, tag="psmm")
                nc.tensor.matmul(ps, lhsT=bd_tiles[hp], rhs=W['wo'][:, hp, :],
                                 start=True, stop=True)
                if hp % 2 == 0:
                    nc.scalar.copy(out=r_sb[:, hp, :], in_=ps)
                else:
                    nc.vector.tensor_copy(out=r_sb[:, hp, :], in_=ps)

        def emit_W2(b):
            """W_effT[c, o] = sum_e wv[e, c] R[e, o], m-outer."""
            r_sb = st_r[b]
            w2 = w2pool.tile([P, KC, C], F16, tag="w2", name=f"w2_{b}")
            st_w2[b] = w2
            for m in range(KC):
                ps = psmm.tile([P, C], F32, tag="psmm")
                for ki in range(KC):
                    nc.tensor.matmul(
                        ps,
                        lhsT=W['wv'][:, ki, m * P:(m + 1) * P],
                        rhs=r_sb[:, ki, :],
                        start=(ki == 0), stop=(ki == KC - 1))
                if m % 2 == 0:
                    nc.scalar.copy(out=w2[:, m, :], in_=ps)
                else:
                    nc.vector.tensor_copy(out=w2[:, m, :], in_=ps)

        st_by = {}

        def emit_By_setup(b):
            y_lo = ypool.tile([P, 2, N], F16, tag="y", name=f"ylo_{b}")
            y_hi = ypool.tile([P, 2, N], F16, tag="y", name=f"yhi_{b}")
            st = stats.tile([P, KC, NS, 6], F32, tag="bnstats")
            mv_t = stats.tile([P, KC, 2], F32, tag="mv")
            st_y[b] = (y_lo, y_hi)
            st_stats[b] = mv_t
            st_by[b] = (st, {})
            for ns in range(2):
                xb = xpool.tile([P, KC, 512], F16, tag="xblk",
                                name=f"x_{b}_{ns}")
                nc.sync.dma_start(
                    out=xb, in_=x_d[b, ns].rearrange("p (k n) -> p k n", k=KC))
                st_by[b][1][ns] = xb

        def emit_By_blocks(b, blocks):
            """y = W_eff @ x (+bias) + bn_stats, streaming x blocks."""
            w2 = st_w2[b]
            y_lo, y_hi = st_y[b]
            st, x_blks = st_by[b]
            for ns in blocks:
                if ns + 2 < NS:
                    xb = xpool.tile([P, KC, 512], F16, tag="xblk",
                                    name=f"x_{b}_{ns + 2}")
                    nc.sync.dma_start(
                        out=xb,
                        in_=x_d[b, ns + 2].rearrange("p (k n) -> p k n", k=KC))
                    x_blks[ns + 2] = xb
                x_blk = x_blks.pop(ns)
                for m in range(KC):
                    yt = y_lo if m < 2 else y_hi
                    mi = m % 2
                    ps = psmm.tile([P, 512], F32, tag="psmm")
                    for k in range(KC):
                        nc.tensor.matmul(
                            ps,
                            lhsT=w2[:, k, m * P:(m + 1) * P],
                            rhs=x_blk[:, k, :],
                            start=(k == 0), stop=(k == KC - 1))
                    # pure-copy psum evacuation (bias folded into the apply
                    # offset); stats read the psum in parallel on DVE.
                    # Split: ACT 3 copies (2.1us) / DVE 1 copy + 4 stats
                    # (2.6us) per 3.4us PE block.
                    ysl = yt[:, mi, ns * 512:(ns + 1) * 512]
                    if m == 1:
                        nc.vector.tensor_copy(out=ysl, in_=ps)
                    else:
                        nc.scalar.copy(out=ysl, in_=ps)
                    nc.vector.bn_stats(out=st[:, m, ns, :], in_=ps)

        def emit_By_aggr(b):
            st, _ = st_by[b]
            mv_t = st_stats[b]
            for m in range(KC):
                nc.vector.bn_aggr(out=mv_t[:, m, :], in_=st[:, m])

        st_ts = {}

        def emit_tail_stats_a(b):
            """per-channel stat fold + cross-partition reduce (PE)."""
            mv = st_stats[b]
            # S[p, stat, m]: 0 = mean+bias, 1 = var, 2 = (mean+bias)^2
            s_t = stats.tile([P, 3, KC], F32, tag="s_t")
            nc.vector.tensor_add(s_t[:, 0, :], mv[:, :, 0], bias_sb)
            nc.vector.tensor_copy(out=s_t[:, 1, :], in_=mv[:, :, 1])
            nc.vector.tensor_mul(s_t[:, 2, :], s_t[:, 0, :], s_t[:, 0, :])
            pstat = psmm.tile([1, 3, KC], F32, tag="psmm")
            nc.tensor.matmul(pstat, lhsT=ones_col, rhs=s_t,
                             start=True, stop=True)
            st_ts[b] = {"pstat": pstat}

        def emit_tail_stats_b(b):
            """scalar chain: mean/var totals + DVE-only rsqrt (Quake bit
            trick + Newton; no ACT => no activation-table swaps)."""
            pstat = st_ts[b]["pstat"]
            e3 = stats.tile([1, 3], F32, tag="e3")
            nc.vector.reduce_sum(out=e3, in_=pstat, axis=AX.X)
            m2 = stats.tile([1, 1], F32, tag="m2")
            nc.vector.tensor_mul(m2, e3[:, 0:1], e3[:, 0:1])
            vb = stats.tile([1, 1], F32, tag="vb")
            nc.vector.tensor_add(vb, e3[:, 1:2], e3[:, 2:3])
            nc.vector.tensor_sub(vb, vb, m2)
            nc.vector.tensor_add(vb, vb, eps_sb)
            sc2 = stats.tile([1, 2], F32, tag="sc2")
            nc.vector.tensor_copy(out=sc2[:, 0:1], in_=e3[:, 0:1])
            hv = stats.tile([1, 1], F32, tag="hv")
            nc.vector.tensor_scalar_mul(hv, vb, 0.5)
            r = stats.tile([1, 1], F32, tag="rq")
            nc.vector.tensor_scalar(
                out=r.bitcast(mybir.dt.int32), in0=vb.bitcast(mybir.dt.int32),
                scalar1=1, scalar2=None, op0=ALU.logical_shift_right)
            nc.vector.tensor_tensor(r.bitcast(mybir.dt.int32), magic_sb,
                                    r.bitcast(mybir.dt.int32), ALU.subtract)
            t1 = stats.tile([1, 1], F32, tag="t1")
            for _ in range(2):
                nc.vector.tensor_mul(t1, r, r)
                nc.vector.tensor_mul(t1, hv, t1)
                nc.vector.tensor_sub(t1, c15_sb, t1)
                nc.vector.tensor_mul(r, r, t1)
            nc.vector.tensor_copy(out=sc2[:, 1:2], in_=r)
            st_ts[b]["sc2"] = sc2

        def emit_tail_stats_c(b):
            """broadcast (PE) + per-channel scale/offset."""
            sc2 = st_ts[b]["sc2"]
            bc_ps = psmm.tile([P, 2], F32, tag="psmm")
            nc.tensor.matmul(bc_ps, lhsT=ones_row, rhs=sc2,
                             start=True, stop=True)
            # s = gamma * rstd ; t = beta + (bias - mean_total) * s
            # (bias folded here so the y psum evacuation is a pure copy)
            s_ch = stats.tile([P, KC], F32, tag="s_ch")
            nc.vector.tensor_scalar_mul(s_ch, gamma_sb, bc_ps[:, 1:2])
            u = stats.tile([P, KC], F32, tag="tb")
            nc.vector.tensor_scalar(
                out=u, in0=bias_sb, scalar1=bc_ps[:, 0:1], scalar2=None,
                op0=ALU.subtract)
            t_ch = stats.tile([P, KC], F32, tag="t_ch")
            nc.vector.tensor_mul(t_ch, u, s_ch)
            nc.vector.tensor_add(t_ch, t_ch, beta_sb)
            st_scale[b] = (s_ch, t_ch)

        def emit_tail_stats(b):
            emit_tail_stats_a(b)
            emit_tail_stats_b(b)
            emit_tail_stats_c(b)

        # apply engine split: DVE applies f16 ~3x faster than ACT, so go
        # DVE-heavy (gpsimd slices measured 2.3us AND slowed the other
        # engines via SBUF port contention -- keep gpsimd out).
        APPLY_ENG = {0: "ddaddadd", 1: "ddaddadd"}

        def emit_tail_apply(b):
            """normalization apply + writeout."""
            y_lo, y_hi = st_y[b]
            s_ch, t_ch = st_scale[b]
            for i, (m, h) in enumerate(
                    (m, h) for m in range(KC) for h in range(2)):
                yt = y_lo if m < 2 else y_hi
                mi = m % 2
                sl = slice(h * (N // 2), (h + 1) * (N // 2))
                eng = APPLY_ENG[b][i]
                if eng == "a":
                    nc.scalar.activation(
                        out=yt[:, mi, sl], in_=yt[:, mi, sl],
                        func=ACTF.Identity,
                        bias=t_ch[:, m:m + 1], scale=s_ch[:, m:m + 1])
                else:
                    e = nc.vector if eng == "d" else nc.gpsimd
                    e.tensor_scalar(
                        out=yt[:, mi, sl], in0=yt[:, mi, sl],
                        scalar1=s_ch[:, m:m + 1], scalar2=t_ch[:, m:m + 1],
                        op0=ALU.mult, op1=ALU.add)
                nc.sync.dma_start(out=out_d[b, m * P:(m + 1) * P, sl],
                                  in_=yt[:, mi, sl])

        # ---- emission schedule (PE queue order is emission order) ----
        emit_G(0, prefetched=XLOOK, hook={
            NT - 1 - XLOOK: lambda: W.update(wk=load_w(wk_d),
                                             wq=load_w(wq_d))})
        W.update(wo=load_w(wo_d), wv=load_w(wv_d))
        emit_T(0)
        emit_scores(0)
        emit_softmax(0)       # DVE, overlaps G1 on PE
        emit_G(1, prefetched=XLOOK)
        emit_R(0)
        emit_W2(0)
        emit_T(1)
        emit_scores(1)
        emit_softmax(1)       # DVE, overlaps y0 on PE
        emit_By_setup(0)
        emit_By_blocks(0, range(NS))
        emit_By_aggr(0)
        emit_R(1)
        emit_W2(1)
        # ts0 is threaded through By1's first blocks: the PE pieces (pstat,
        # bc) land right after W21/blk1 so the DVE chain is never waited on
        emit_tail_stats_a(0)
        emit_tail_stats_b(0)
        emit_By_setup(1)
        emit_By_blocks(1, range(2))
        emit_tail_stats_c(0)
        emit_tail_apply(0)    # gpsimd-heavy, overlaps y1 on PE
        emit_By_blocks(1, range(2, NS))
        emit_By_aggr(1)
        emit_tail_stats(1)
        emit_tail_apply(1)

    nc.finalize()
    return nc


_NC_CACHE = {}


def _get_nc():
    if "nc" not in _NC_CACHE:
        _NC_CACHE["nc"] = build_nc()
    return _NC_CACHE["nc"]


def _prep_w(w):
    # [C_in, C_out] -> [128, KC, C_out] fp16 with c_in = k*128 + p
    return np.ascontiguousarray(
        w.reshape(KC, P, C).transpose(1, 0, 2).astype(np.float16))


def _prep_vec(v):
    # [C] -> [128, KC] with c = k*128 + p
    return np.ascontiguousarray(v.reshape(KC, P).T)


def _prep_x(x):
    # [B, C, N] -> [B, NB, P, KC*512] fp16: block j, partition p, (k, n)
    nb = x.shape[0]
    xr = x.reshape(nb, KC, P, NB, 512)
    return np.ascontiguousarray(
        xr.transpose(0, 3, 2, 1, 4).astype(np.float16)).reshape(
        nb, NB, P, KC * 512)


def _prep_xt(x):
    # [B, C, N] -> [B, NT, P, 2*512] fp16: xt[b,t,p,j*512+c] = x[b,c,(2t+j)*128+p]
    nb = x.shape[0]
    xr = x.reshape(nb, C, NT, 2, P)           # [b, c, t, j, p]
    return np.ascontiguousarray(
        xr.transpose(0, 2, 4, 3, 1).astype(np.float16)).reshape(
        nb, NT, P, 1024)


def _make_in_maps(x, w_qkv, w_out, b_out, gamma, beta):
    x = np.asarray(x, dtype=np.float32).reshape(B, C, N)
    xr = _prep_x(x)
    xtr = _prep_xt(x)
    w_qkv = np.asarray(w_qkv, dtype=np.float32)
    wq = _prep_w(np.ascontiguousarray(w_qkv[0:C].T))
    wk = _prep_w(np.ascontiguousarray(w_qkv[C:2 * C].T))
    wv = _prep_w(np.ascontiguousarray(w_qkv[2 * C:3 * C]))
    wo = _prep_w(np.ascontiguousarray(np.asarray(w_out, dtype=np.float32).T))
    bvec = _prep_vec(np.asarray(b_out, dtype=np.float32))
    gam = _prep_vec(np.asarray(gamma, dtype=np.float32))
    bet = _prep_vec(np.asarray(beta, dtype=np.float32))
    return [
        dict(x=np.ascontiguousarray(xr[c * PB:(c + 1) * PB]),
             xt=np.ascontiguousarray(xtr[c * PB:(c + 1) * PB]),
             wq=wq, wk=wk, wv=wv, wo=wo,
             bvec=bvec, gamma=gam, beta=bet)
        for c in range(NCORES)
    ]


def _run(inputs, trace=False, trace_kwargs=None):
    nc = _get_nc()
    in_maps = _make_in_maps(**inputs)
    res = run_bass_kernel_spmd(nc, in_maps, core_ids=list(range(NCORES)),
                               trace=trace, **(trace_kwargs or {}))
    out = np.concatenate([res.results[c]["out"].astype(np.float32)
                          for c in range(NCORES)], axis=0)
    return out.reshape(B, C, HW_SIDE, HW_SIDE), res


def kernel(x, w_qkv, w_out, b_out, gamma, beta):
    inputs = dict(x=x, w_qkv=w_qkv, w_out=w_out, b_out=b_out,
                  gamma=gamma, beta=beta)
    try:
        out, _ = _run(inputs)
    except Exception:
        # transient device errors (e.g. NRT_EXEC_UNIT_UNRECOVERABLE) have
        # been observed once across many runs; one retry recovers.
        out, _ = _run(inputs)
    return out


# revision 40
# speedup vs baseline: 1.0012x; 1.0012x over previous
"""Trainium2 Bass kernel for nn_MultiHeadAttention_63814624084186.

Reference computation (per batch sample b, fully independent across b):
  x: [512, 4096]  (C channels x N=64*64 pixels)
  qkv = w_qkv @ x            -> q,k,v each [512, 4096] (8 heads x 64 dims)
  scores = (q_h @ k_h^T)/8   -> [64, 64] per head   (channel-attention)
  attn = softmax(scores, -1)
  out_h = attn_h @ v_h       -> [64, 4096]
  y = w_out @ out + b_out    -> [512, 4096]
  y = groupnorm(y over all C,N) * gamma + beta

Key algebra (this version): attention is over the CHANNEL dim, so
  scores_h = q_h k_h^T = (w_q G w_k^T)_h   with  G = x x^T  [512,512]
  y = w_out bd(A) w_v x = W_eff x          with  W_eff folded on-chip
q, k, v are never materialized.  Per-batch PE work drops from ~4.5e9
MACs (qkv + v + out-proj) to ~2.4e9 (G + y GEMM + small folds).

GroupNorm stats are computed ANALYTICALLY before the y GEMM runs:
  sum(y)   = 1^T W_eff xs        (xs = sum_n x, host-prepared)
  sum(y^2) = tr((W_eff^T W_eff) G) = <M, G>   (G already on chip)
so the per-channel scale/offset are known up front, the normalization is
fused into the y-psum evacuation, and each output tile is DMA'd to HBM
as soon as it is produced: no bn_stats, no apply pass, no output drain.

Sharding: pure data-parallel over batch: 16 samples / 8 cores = 2 per core.

Pipeline (PE queue order; b0/b1 are the two per-core batches):
  G0 T0 sc0 | G1 | R0 W20 M0 | T1 sc1 (stat mms b0) | By0[0:6] |
  R1 W21 M1 | By0[6:8] (stat mms b1) | By1
softmax/stat chains run on DVE under the covering PE phases.
"""

import numpy as np
from contextlib import ExitStack

import concourse.bass as bass
import concourse.tile as tile
from concourse import bacc, mybir
from concourse.bass_utils import run_bass_kernel_spmd
from concourse.masks import make_identity

F32 = mybir.dt.float32
F16 = mybir.dt.float16
I32 = mybir.dt.int32
AX = mybir.AxisListType
ALU = mybir.AluOpType
ACTF = mybir.ActivationFunctionType

B = 16          # global batch
C = 512         # channels
N = 4096        # pixels (64*64)
HW_SIDE = 64
NCORES = 8
PB = B // NCORES  # batches per core
P = 128
KC = C // P     # 4 channel chunks
NB = 8          # n blocks of 512 (y GEMM)
NT = 16         # xT tiles of 2 n-chunks each (G GEMM)
NS = N // 512   # 8 pixel chunks of 512
NHP = 4         # head pairs
XLOOK = 8       # xT DMA lookahead tiles
EPS = 1e-5
CN = C * N


def build_nc():
    nc = bacc.Bacc("TRN2", target_bir_lowering=False, debug=False,
                   num_devices=NCORES)

    # xT[b, t, p, j*512+c] = x[b, c, (2t+j)*128 + p]
    xt_d = nc.declare_dram_parameter("xt", [PB, NT, P, 1024], F16, isOutput=False)
    # x[b, nb, p, k*512+n] = x[b, k*128+p, nb*512+n]
    x_d = nc.declare_dram_parameter("x", [PB, NB, P, KC * 512], F16, isOutput=False)
    wq_d = nc.declare_dram_parameter("wq", [P, KC, C], F16, isOutput=False)   # w_q^T
    wk_d = nc.declare_dram_parameter("wk", [P, KC, C], F16, isOutput=False)   # w_k^T
    wv_d = nc.declare_dram_parameter("wv", [P, KC, C], F16, isOutput=False)   # w_v
    wo_d = nc.declare_dram_parameter("wo", [P, KC, C], F16, isOutput=False)   # w_out^T
    xs_d = nc.declare_dram_parameter("xs", [P, PB, KC], F16, isOutput=False)
    bias_d = nc.declare_dram_parameter("bvec", [P, KC], F32, isOutput=False)
    gamma_d = nc.declare_dram_parameter("gamma", [P, KC], F32, isOutput=False)
    beta_d = nc.declare_dram_parameter("beta", [P, KC], F32, isOutput=False)
    bconst_d = nc.declare_dram_parameter("bconst", [1, 2], F32, isOutput=False)
    out_d = nc.declare_dram_parameter("out", [PB, C, N], F16, isOutput=True)

    with tile.TileContext(nc) as tc, ExitStack() as ctx:
        consts = ctx.enter_context(tc.tile_pool(name="consts", bufs=1))
        xtpool = ctx.enter_context(tc.tile_pool(name="xtpool", bufs=XLOOK + 2))
        xpool = ctx.enter_context(tc.tile_pool(name="xpool", bufs=10))
        gpool = ctx.enter_context(tc.tile_pool(name="gpool", bufs=2))
        tpool = ctx.enter_context(tc.tile_pool(name="tpool", bufs=2))
        rpool = ctx.enter_context(tc.tile_pool(name="rpool", bufs=2))
        w2pool = ctx.enter_context(tc.tile_pool(name="w2pool", bufs=2))
        w2tpool = ctx.enter_context(tc.tile_pool(name="w2tpool", bufs=2))
        mpool = ctx.enter_context(tc.tile_pool(name="mpool", bufs=2))
        ybuf = ctx.enter_context(tc.tile_pool(name="ybuf", bufs=10))
        attn = ctx.enter_context(tc.tile_pool(name="attn", bufs=8))
        attnt = ctx.enter_context(tc.tile_pool(name="attnt", bufs=4))
        stats = ctx.enter_context(tc.tile_pool(name="stats", bufs=4))
        # psg serves G (4 full banks), the scores tiles and the w2-transpose
        # staging: a matmul start=True resets the target bank's whole
        # per-partition row, so concurrent accumulators need separate banks
        # (partition packing 0:64/64:128 within a bank is safe, free-offset
        # packing is NOT).
        psg = ctx.enter_context(tc.tile_pool(name="psg", bufs=4, space="PSUM"))
        psmm = ctx.enter_context(tc.tile_pool(name="psmm", bufs=4, space="PSUM"))

        def load_w(dram):
            t = consts.tile([P, KC, C], F16, tag=f"w_{dram.name}")
            nc.sync.dma_start(out=t, in_=dram[:, :, :])
            return t

        xt_tiles = {}

        def fetch_xt(b, t):
            xt = xtpool.tile([P, 2, 512], F16, tag="xt", name=f"xt_{b}_{t}")
            nc.sync.dma_start(
                out=xt, in_=xt_d[b, t].rearrange("p (j c) -> p j c", j=2))
            xt_tiles[(b, t)] = xt

        # xT tiles first: weight loads are deferred until G0's xT stream is
        # fully issued (weights are only needed from T0 onward), so the
        # first G matmuls are never DMA-supply-gated.
        for t in range(XLOOK):
            fetch_xt(0, t)
        W = {}

        xs_sb = consts.tile([P, PB, KC], F16, tag="xs")
        nc.gpsimd.dma_start(out=xs_sb, in_=xs_d[:, :, :])
        bias_sb = consts.tile([P, KC], F32, tag="bias")
        nc.gpsimd.dma_start(out=bias_sb, in_=bias_d[:, :])
        gamma_sb = consts.tile([P, KC], F32, tag="gamma")
        nc.gpsimd.dma_start(out=gamma_sb, in_=gamma_d[:, :])
        beta_sb = consts.tile([P, KC], F32, tag="beta")
        nc.gpsimd.dma_start(out=beta_sb, in_=beta_d[:, :])
        bconst_sb = consts.tile([1, 2], F32, tag="bconst")
        nc.gpsimd.dma_start(out=bconst_sb, in_=bconst_d[:, :])

        ident_sb = consts.tile([P, P], F16, tag="ident")
        make_identity(nc, ident_sb)
        eps_sb = consts.tile([1, 1], F32, tag="eps")
        nc.vector.memset(eps_sb, EPS)
        # pre-warm the exp activation table (softmax) so no ACT table load
        # lands mid-kernel; the stats chain's rsqrt runs DVE-only.
        warm_sb = consts.tile([1, 1], F32, tag="warm")
        nc.scalar.activation(out=warm_sb, in_=eps_sb, func=ACTF.Exp,
                             bias=0.0, scale=0.0)
        magic_sb = consts.tile([1, 1], I32, tag="magic")
        nc.vector.memset(magic_sb, 0x5f3759df)
        c15_sb = consts.tile([1, 1], F32, tag="c15")
        nc.vector.memset(c15_sb, 1.5)
        ones_col = consts.tile([P, 1], F32, tag="ones_col")
        nc.vector.memset(ones_col, 1.0)
        ones_row = consts.tile([1, P], F32, tag="ones_row")
        nc.vector.memset(ones_row, 1.0)

        # per-batch state carried between emission stages
        st_g = {}    # G in SBUF (f16) [P, KC, C]
        st_t = {}    # T = G @ wk^T   [P, KC, C]
        st_sc = {}   # scores psum tiles
        st_at = {}   # block-diag attn tiles
        st_r = {}    # R = bd(A)^T @ wo^T
        st_w2 = {}   # W_effT = wv^T-contract @ R
        st_ts = {}   # analytic-stats intermediates
        st_scale = {}
        st_by = {}

        def emit_G(b, prefetched, hook=None):
            """G = x x^T, t-outer: each xT tile is consumed then retired.
            Only the upper block-triangle is computed (rhs = cols >= m*128);
            the 6 lower [128,128] blocks are PE-transposed from the upper
            copies.  All 4 chunk accumulators live in 4 psum banks."""
            g_sb = gpool.tile([P, KC, C], F16, tag="g", name=f"g_{b}")
            st_g[b] = g_sb
            ps = [psg.tile([P, C - m * P], F32, tag="psg", name=f"g_{b}_{m}")
                  for m in range(KC)]
            for t in range(NT):
                tf = t + prefetched
                if tf < NT:
                    fetch_xt(b, tf)
                elif b + 1 < PB and tf - NT < NT:
                    fetch_xt(b + 1, tf - NT)
                if hook and t in hook:
                    hook[t]()
                xt = xt_tiles.pop((b, t))
                for j in range(2):
                    for m in range(KC):
                        nc.tensor.matmul(
                            ps[m],
                            lhsT=xt[:, j, m * P:(m + 1) * P],
                            rhs=xt[:, j, m * P:],
                            start=(t == 0 and j == 0),
                            stop=(t == NT - 1 and j == 1),
                            skip_group_check=True)
            for m in range(KC):
                if m % 2 == 0:
                    nc.scalar.copy(out=g_sb[:, m, m * P:], in_=ps[m])
                else:
                    nc.vector.tensor_copy(out=g_sb[:, m, m * P:], in_=ps[m])
            # lower blocks (m, mp<m) = transpose(upper block (mp, m)),
            # ordered so T chunk 0's operands are ready first
            for m, mp in ((1, 0), (2, 0), (3, 0), (2, 1), (3, 1), (3, 2)):
                pst = psmm.tile([P, P], F16, tag="psmm")
                nc.tensor.transpose(
                    pst, g_sb[:, mp, m * P:(m + 1) * P], ident_sb)
                if (m + mp) % 2 == 0:
                    nc.vector.tensor_copy(
                        out=g_sb[:, m, mp * P:(mp + 1) * P], in_=pst)
                else:
                    nc.scalar.copy(
                        out=g_sb[:, m, mp * P:(mp + 1) * P], in_=pst)

        def emit_T(b):
            """T = G @ wk^T  [c, e], m-outer single-bank accumulation."""
            g_sb = st_g[b]
            t_sb = tpool.tile([P, KC, C], F16, tag="t", name=f"t_{b}")
            st_t[b] = t_sb
            for m in range(KC):
                ps = psmm.tile([P, C], F32, tag="psmm")
                for mp in range(KC):
                    nc.tensor.matmul(
                        ps,
                        lhsT=g_sb[:, mp, m * P:(m + 1) * P],
                        rhs=W['wk'][:, mp, :],
                        start=(mp == 0), stop=(mp == KC - 1))
                nc.vector.tensor_copy(out=t_sb[:, m, :], in_=ps)

        def emit_scores(b):
            """scores_h = (w_q T)_h, two heads packed per psum tile; k-outer
            so the first matmuls only need T chunk 0."""
            t_sb = st_t[b]
            sc_ps = [psg.tile([P, 64], F32, tag="psg", name=f"sc_{b}_{hp}")
                     for hp in range(NHP)]
            st_sc[b] = sc_ps
            for k in range(KC):
                for hp in range(NHP):
                    hA, hB = 2 * hp, 2 * hp + 1
                    clA = slice(hA * 64, hA * 64 + 64)
                    clB = slice(hB * 64, hB * 64 + 64)
                    nc.tensor.matmul(
                        sc_ps[hp][0:64, :],
                        lhsT=W['wq'][:, k, clA], rhs=t_sb[:, k, clA],
                        start=(k == 0), stop=(k == KC - 1),
                        skip_group_check=True)
                    nc.tensor.matmul(
                        sc_ps[hp][64:P, :],
                        lhsT=W['wq'][:, k, clB], rhs=t_sb[:, k, clB],
                        start=(k == 0), stop=(k == KC - 1),
                        skip_group_check=True)

        def emit_softmax(b):
            """softmax over scores (all head pairs batched) -> blockdiag tiles."""
            sc_ps = st_sc[b]
            a_all = attn.tile([P, NHP, 64], F32, tag="a_all")
            for hp in range(NHP):
                nc.vector.tensor_copy(out=a_all[:, hp, :], in_=sc_ps[hp])
            mx = attn.tile([P, NHP, 1], F32, tag="mx4")
            nc.vector.reduce_max(out=mx, in_=a_all, axis=AX.X)
            d_all = attn.tile([P, NHP, 64], F32, tag="d_all")
            nc.vector.tensor_tensor(d_all, a_all,
                                    mx.to_broadcast([P, NHP, 64]), ALU.subtract)
            e_all = attn.tile([P, NHP, 64], F32, tag="e_all")
            nc.scalar.activation(out=e_all, in_=d_all, func=ACTF.Exp,
                                 bias=0.0, scale=0.125)
            sm = attn.tile([P, NHP, 1], F32, tag="sm4")
            nc.vector.reduce_sum(out=sm, in_=e_all, axis=AX.X)
            rs = attn.tile([P, NHP, 1], F32, tag="rs4")
            nc.vector.reciprocal(out=rs, in_=sm)
            a_mm = attn.tile([P, NHP, 64], F16, tag="amm4")
            nc.vector.tensor_tensor(a_mm, e_all,
                                    rs.to_broadcast([P, NHP, 64]), ALU.mult)
            bd_tiles = []
            for hp in range(NHP):
                at = attnt.tile([P, P], F16, tag="attnT", name=f"at_{b}_{hp}")
                nc.gpsimd.memset(at, 0.0)
                nc.vector.tensor_copy(out=at[0:64, 0:64], in_=a_mm[0:64, hp, :])
                nc.vector.tensor_copy(out=at[64:P, 64:P], in_=a_mm[64:P, hp, :])
                bd_tiles.append(at)
            st_at[b] = bd_tiles

        def emit_R(b):
            """R[e, o] = sum_d bd(A)[d, e] wo^T[d, o]."""
            bd_tiles = st_at[b]
            r_sb = rpool.tile([P, KC, C], F16, tag="r", name=f"r_{b}")
            st_r[b] = r_sb
            for hp in range(NHP):
                ps = psmm.tile([P, C], F32, tag="psmm")
                nc.tensor.matmul(ps, lhsT=bd_tiles[hp], rhs=W['wo'][:, hp, :],
                                 start=True, stop=True)
                if hp % 2 == 0:
                    nc.scalar.copy(out=r_sb[:, hp, :], in_=ps)
                else:
                    nc.vector.tensor_copy(out=r_sb[:, hp, :], in_=ps)

        def emit_W2(b):
            """W_effT[c, o] = sum_e wv[e, c] R[e, o], m-outer."""
            r_sb = st_r[b]
            w2 = w2pool.tile([P, KC, C], F16, tag="w2", name=f"w2_{b}")
            st_w2[b] = w2
            for m in range(KC):
                ps = psmm.tile([P, C], F32, tag="psmm")
                for ki in range(KC):
                    nc.tensor.matmul(
                        ps,
                        lhsT=W['wv'][:, ki, m * P:(m + 1) * P],
                        rhs=r_sb[:, ki, :],
                        start=(ki == 0), stop=(ki == KC - 1))
                if m % 2 == 0:
                    nc.scalar.copy(out=w2[:, m, :], in_=ps)
                else:
                    nc.vector.tensor_copy(out=w2[:, m, :], in_=ps)

        def emit_M(b):
            """M = W_eff^T W_eff (upper block-triangle) + ws = W_eff @ xs.
            w2 is transposed on the PE (mo-outer so the M accumulation can
            chase the transpose copies), then M accumulates like G."""
            w2 = st_w2[b]
            w2t = w2tpool.tile([P, KC, C], F16, tag="w2t", name=f"w2t_{b}")
            for mo in range(KC):
                for k in range(KC):
                    pst = psg.tile([P, P], F16, tag="psg")
                    nc.tensor.transpose(
                        pst, w2[:, k, mo * P:(mo + 1) * P], ident_sb)
                    if k % 2 == 0:
                        nc.scalar.copy(
                            out=w2t[:, mo, k * P:(k + 1) * P], in_=pst)
                    else:
                        nc.vector.tensor_copy(
                            out=w2t[:, mo, k * P:(k + 1) * P], in_=pst)
            m_sb = mpool.tile([P, KC, C], F16, tag="m", name=f"m_{b}")
            ps = [psmm.tile([P, C - m * P], F32, tag="psmm", name=f"M_{b}_{m}")
                  for m in range(KC)]
            for mo in range(KC):
                for m in range(KC):
                    nc.tensor.matmul(
                        ps[m],
                        lhsT=w2t[:, mo, m * P:(m + 1) * P],
                        rhs=w2t[:, mo, m * P:],
                        start=(mo == 0), stop=(mo == KC - 1),
                        skip_group_check=True)
            for m in range(KC):
                if m % 2 == 0:
                    nc.scalar.copy(out=m_sb[:, m, m * P:], in_=ps[m])
                else:
                    nc.vector.tensor_copy(out=m_sb[:, m, m * P:], in_=ps[m])
            # ws[o] = sum_c W_eff[o, c] xs[c]
            ws_sb = stats.tile([P, KC], F32, tag="ws")
            for m in range(KC):
                wps = psmm.tile([P, 1], F32, tag="psmm")
                for k in range(KC):
                    nc.tensor.matmul(
                        wps,
                        lhsT=w2[:, k, m * P:(m + 1) * P],
                        rhs=xs_sb[:, b, k:k + 1],
                        start=(k == 0), stop=(k == KC - 1))
                nc.vector.tensor_copy(out=ws_sb[:, m:m + 1], in_=wps)
            st_ts[b] = {"m_sb": m_sb, "ws": ws_sb}

        def emit_stats_a(b):
            """DVE: tr(M G) partials (diag first, then uppers x2) + ws sums
            -> svec [P, 3] ready for the cross-partition reduce."""
            m_sb = st_ts[b]["m_sb"]
            ws = st_ts[b]["ws"]
            g_sb = st_g[b]
            trp = stats.tile([P, 10], F32, tag="trp")
            scr = stats.tile([P, P], F32, tag="scr")
            blocks = [(m, m) for m in range(KC)] + \
                     [(m, mp) for m in range(KC) for mp in range(m + 1, KC)]
            for idx, (m, mp) in enumerate(blocks):
                off = mp * P
                nc.vector.tensor_tensor(
                    scr, m_sb[:, m, off:off + P],
                    g_sb[:, m, off:off + P], ALU.mult)
                nc.vector.reduce_sum(out=trp[:, idx:idx + 1], in_=scr,
                                     axis=AX.X)
            sv = stats.tile([P, 3], F32, tag="sv")
            dd = stats.tile([P, 2], F32, tag="dd")
            nc.vector.reduce_sum(out=dd[:, 0:1], in_=trp[:, 0:KC], axis=AX.X)
            nc.vector.reduce_sum(out=dd[:, 1:2], in_=trp[:, KC:10], axis=AX.X)
            nc.vector.tensor_scalar(
                out=sv[:, 0:1], in0=dd[:, 1:2], scalar1=2.0, scalar2=None,
                op0=ALU.mult)
            nc.vector.tensor_add(sv[:, 0:1], sv[:, 0:1], dd[:, 0:1])
            u = stats.tile([P, KC], F32, tag="u")
            nc.vector.tensor_mul(u, ws, bias_sb)
            nc.vector.reduce_sum(out=sv[:, 1:2], in_=ws, axis=AX.X)
            nc.vector.reduce_sum(out=sv[:, 2:3], in_=u, axis=AX.X)
            st_ts[b]["sv"] = sv

        def emit_stats_mm(b):
            """cross-partition reduce of [tr, S1, S2] (PE)."""
            sv = st_ts[b]["sv"]
            p3 = psmm.tile([1, 3], F32, tag="psmm", name=f"p3_{b}")
            nc.tensor.matmul(p3, lhsT=ones_col, rhs=sv, start=True, stop=True)
            st_ts[b]["p3"] = p3

        def emit_stats_b(b):
            """scalar combine: mu, var, rstd (DVE-only quake rsqrt)."""
            p3 = st_ts[b]["p3"]
            sc2 = stats.tile([1, 2], F32, tag="sc2")
            # mu = S1/CN + B1/C
            nc.vector.tensor_scalar(
                out=sc2[:, 0:1], in0=p3[:, 1:2], scalar1=1.0 / CN,
                scalar2=None, op0=ALU.mult)
            nc.vector.tensor_add(sc2[:, 0:1], sc2[:, 0:1], bconst_sb[:, 0:1])
            # E2 = TR/CN + 2*S2/CN + B2/C ; var = E2 - mu^2 + eps
            vb = stats.tile([1, 1], F32, tag="vb")
            nc.vector.tensor_scalar(
                out=vb, in0=p3[:, 2:3], scalar1=2.0 / CN, scalar2=None,
                op0=ALU.mult)
            t2 = stats.tile([1, 1], F32, tag="t2")
            nc.vector.tensor_scalar(
                out=t2, in0=p3[:, 0:1], scalar1=1.0 / CN, scalar2=None,
                op0=ALU.mult)
            nc.vector.tensor_add(vb, vb, t2)
            nc.vector.tensor_add(vb, vb, bconst_sb[:, 1:2])
            m2 = stats.tile([1, 1], F32, tag="m2")
            nc.vector.tensor_mul(m2, sc2[:, 0:1], sc2[:, 0:1])
            nc.vector.tensor_sub(vb, vb, m2)
            nc.vector.tensor_add(vb, vb, eps_sb)
            # rstd = 1/sqrt(vb): quake bit trick + 2 Newton steps (DVE only)
            hv = stats.tile([1, 1], F32, tag="hv")
            nc.vector.tensor_scalar_mul(hv, vb, 0.5)
            r = stats.tile([1, 1], F32, tag="rq")
            nc.vector.tensor_scalar(
                out=r.bitcast(I32), in0=vb.bitcast(I32),
                scalar1=1, scalar2=None, op0=ALU.logical_shift_right)
            nc.vector.tensor_tensor(r.bitcast(I32), magic_sb,
                                    r.bitcast(I32), ALU.subtract)
            t1 = stats.tile([1, 1], F32, tag="t1")
            for _ in range(2):
                nc.vector.tensor_mul(t1, r, r)
                nc.vector.tensor_mul(t1, hv, t1)
                nc.vector.tensor_sub(t1, c15_sb, t1)
                nc.vector.tensor_mul(r, r, t1)
            nc.vector.tensor_copy(out=sc2[:, 1:2], in_=r)
            st_ts[b]["sc2"] = sc2

        def emit_stats_bc(b):
            """broadcast mu/rstd to all partitions (PE)."""
            sc2 = st_ts[b]["sc2"]
            bc_ps = psmm.tile([P, 2], F32, tag="psmm", name=f"bc_{b}")
            nc.tensor.matmul(bc_ps, lhsT=ones_row, rhs=sc2,
                             start=True, stop=True)
            st_ts[b]["bc"] = bc_ps

        def emit_stats_c(b):
            """per-channel scale/offset: s = gamma*rstd,
            t = beta + (bias - mu)*s."""
            bc_ps = st_ts[b]["bc"]
            s_ch = stats.tile([P, KC], F32, tag="s_ch")
            nc.vector.tensor_scalar_mul(s_ch, gamma_sb, bc_ps[:, 1:2])
            u = stats.tile([P, KC], F32, tag="tb")
            nc.vector.tensor_scalar(
                out=u, in0=bias_sb, scalar1=bc_ps[:, 0:1], scalar2=None,
                op0=ALU.subtract)
            t_ch = stats.tile([P, KC], F32, tag="t_ch")
            nc.vector.tensor_mul(t_ch, u, s_ch)
            nc.vector.tensor_add(t_ch, t_ch, beta_sb)
            st_scale[b] = (s_ch, t_ch)

        def emit_By_setup(b):
            x_blks = {}
            for ns in range(NS):
                xb = xpool.tile([P, KC, 512], F16, tag="xblk",
                                name=f"x_{b}_{ns}")
                nc.sync.dma_start(
                    out=xb, in_=x_d[b, ns].rearrange("p (k n) -> p k n", k=KC))
                x_blks[ns] = xb
            st_by[b] = (x_blks, {})

        def emit_By_blocks(b, blocks):
            """y = W_eff @ x with the groupnorm apply FUSED into the psum
            evacuation (s/t known up front); finished [P,1024] output tiles
            are DMA'd to HBM immediately."""
            w2 = st_w2[b]
            s_ch, t_ch = st_scale[b]
            x_blks, ybufs = st_by[b]
            for ns in blocks:
                pair, half = ns // 2, ns % 2
                x_blk = x_blks.pop(ns)
                for m in range(KC):
                    if half == 0:
                        yb = ybuf.tile([P, 1024], F16, tag="yb",
                                       name=f"yb_{b}_{m}_{pair}")
                        ybufs[(m, pair)] = yb
                    else:
                        yb = ybufs[(m, pair)]
                    ps = psmm.tile([P, 512], F32, tag="psmm")
                    for k in range(KC):
                        nc.tensor.matmul(
                            ps,
                            lhsT=w2[:, k, m * P:(m + 1) * P],
                            rhs=x_blk[:, k, :],
                            start=(k == 0), stop=(k == KC - 1))
                    ysl = yb[:, half * 512:(half + 1) * 512]
                    if m % 2 == 1:
                        nc.vector.tensor_scalar(
                            out=ysl, in0=ps,
                            scalar1=s_ch[:, m:m + 1], scalar2=t_ch[:, m:m + 1],
                            op0=ALU.mult, op1=ALU.add)
                    else:
                        nc.scalar.activation(
                            out=ysl, in_=ps, func=ACTF.Identity,
                            bias=t_ch[:, m:m + 1], scale=s_ch[:, m:m + 1])
                    if half == 1:
                        nc.sync.dma_start(
                            out=out_d[b, m * P:(m + 1) * P,
                                      pair * 1024:(pair + 1) * 1024],
                            in_=yb)

        # ---- emission schedule (PE queue order is emission order) ----
        emit_G(0, prefetched=XLOOK, hook={
            NT - 1 - XLOOK: lambda: W.update(wk=load_w(wk_d),
                                             wq=load_w(wq_d))})
        W.update(wo=load_w(wo_d), wv=load_w(wv_d))
        emit_T(0)
        emit_scores(0)
        emit_softmax(0)       # DVE, overlaps G1 on PE
        emit_G(1, prefetched=XLOOK)
        emit_R(0)
        emit_W2(0)
        emit_M(0)
        emit_stats_a(0)       # DVE, overlaps T1 on PE
        emit_T(1)
        emit_stats_mm(0)
        emit_stats_b(0)       # DVE, overlaps sc1 on PE
        emit_scores(1)
        emit_stats_bc(0)
        emit_stats_c(0)
        emit_softmax(1)       # DVE, overlaps By0 on PE
        emit_By_setup(0)
        emit_By_blocks(0, range(6))
        emit_R(1)
        emit_W2(1)
        emit_M(1)
        emit_stats_a(1)       # DVE, overlaps By0 tail on PE
        emit_By_blocks(0, range(6, 7))
        emit_stats_mm(1)
        emit_stats_b(1)
        emit_By_blocks(0, range(7, NS))
        emit_stats_bc(1)
        emit_stats_c(1)
        emit_By_setup(1)
        emit_By_blocks(1, range(NS))

    nc.finalize()
    return nc


_NC_CACHE = {}


def _get_nc():
    if "nc" not in _NC_CACHE:
        _NC_CACHE["nc"] = build_nc()
    return _NC_CACHE["nc"]


def _prep_w(w):
    # [C_in, C_out] -> [128, KC, C_out] fp16 with c_in = k*128 + p
    return np.ascontiguousarray(
        w.reshape(KC, P, C).transpose(1, 0, 2).astype(np.float16))


def _prep_vec(v):
    # [C] -> [128, KC] with c = k*128 + p
    return np.ascontiguousarray(v.reshape(KC, P).T)


def _prep_x(x):
    # [B, C, N] -> [B, NB, P, KC*512] fp16: block j, partition p, (k, n)
    nb = x.shape[0]
    xr = x.reshape(nb, KC, P, NB, 512)
    return np.ascontiguousarray(
        xr.transpose(0, 3, 2, 1, 4).astype(np.float16)).reshape(
        nb, NB, P, KC * 512)


def _prep_xt(x):
    # [B, C, N] -> [B, NT, P, 2*512] fp16: xt[b,t,p,j*512+c] = x[b,c,(2t+j)*128+p]
    nb = x.shape[0]
    xr = x.reshape(nb, C, NT, 2, P)           # [b, c, t, j, p]
    return np.ascontiguousarray(
        xr.transpose(0, 2, 4, 3, 1).astype(np.float16)).reshape(
        nb, NT, P, 1024)


def _make_in_maps(x, w_qkv, w_out, b_out, gamma, beta):
    x = np.asarray(x, dtype=np.float32).reshape(B, C, N)
    xr = _prep_x(x)
    xtr = _prep_xt(x)
    # xs[p, b, k] = sum_n x[b, k*128+p, n]
    xs = x.sum(axis=2).reshape(B, KC, P).transpose(2, 0, 1)
    xs = np.ascontiguousarray(xs.astype(np.float16))
    w_qkv = np.asarray(w_qkv, dtype=np.float32)
    wq = _prep_w(np.ascontiguousarray(w_qkv[0:C].T))
    wk = _prep_w(np.ascontiguousarray(w_qkv[C:2 * C].T))
    wv = _prep_w(np.ascontiguousarray(w_qkv[2 * C:3 * C]))
    wo = _prep_w(np.ascontiguousarray(np.asarray(w_out, dtype=np.float32).T))
    b_out = np.asarray(b_out, dtype=np.float32)
    bvec = _prep_vec(b_out)
    gam = _prep_vec(np.asarray(gamma, dtype=np.float32))
    bet = _prep_vec(np.asarray(beta, dtype=np.float32))
    bconst = np.array([[b_out.sum() / C, (b_out * b_out).sum() / C]],
                      dtype=np.float32)
    return [
        dict(x=np.ascontiguousarray(xr[c * PB:(c + 1) * PB]),
             xt=np.ascontiguousarray(xtr[c * PB:(c + 1) * PB]),
             xs=np.ascontiguousarray(xs[:, c * PB:(c + 1) * PB]),
             wq=wq, wk=wk, wv=wv, wo=wo,
             bvec=bvec, gamma=gam, beta=bet, bconst=bconst)
        for c in range(NCORES)
    ]


def _run(inputs, trace=False, trace_kwargs=None):
    nc = _get_nc()
    in_maps = _make_in_maps(**inputs)
    res = run_bass_kernel_spmd(nc, in_maps, core_ids=list(range(NCORES)),
                               trace=trace, **(trace_kwargs or {}))
    out = np.concatenate([res.results[c]["out"].astype(np.float32)
                          for c in range(NCORES)], axis=0)
    return out.reshape(B, C, HW_SIDE, HW_SIDE), res


def kernel(x, w_qkv, w_out, b_out, gamma, beta):
    inputs = dict(x=x, w_qkv=w_qkv, w_out=w_out, b_out=b_out,
                  gamma=gamma, beta=beta)
    try:
        out, _ = _run(inputs)
    except Exception:
        # transient device errors (e.g. NRT_EXEC_UNIT_UNRECOVERABLE) have
        # been observed once across many runs; one retry recovers.
        out, _ = _run(inputs)
    return out


# revision 41
# speedup vs baseline: 1.0523x; 1.0510x over previous
"""Trainium2 Bass kernel for nn_MultiHeadAttention_63814624084186.

Reference computation (per batch sample b, fully independent across b):
  x: [512, 4096]  (C channels x N=64*64 pixels)
  qkv = w_qkv @ x            -> q,k,v each [512, 4096] (8 heads x 64 dims)
  scores = (q_h @ k_h^T)/8   -> [64, 64] per head   (channel-attention)
  attn = softmax(scores, -1)
  out_h = attn_h @ v_h       -> [64, 4096]
  y = w_out @ out + b_out    -> [512, 4096]
  y = groupnorm(y over all C,N) * gamma + beta

Key algebra (this version): attention is over the CHANNEL dim, so
  scores_h = q_h k_h^T = (w_q G w_k^T)_h   with  G = x x^T  [512,512]
  y = w_out bd(A) w_v x = W_eff x          with  W_eff folded on-chip
q, k, v are never materialized.  Per-batch PE work drops from ~4.5e9
MACs (qkv + v + out-proj) to ~2.4e9 (G + y GEMM + small folds).

GroupNorm stats are computed ANALYTICALLY before the y GEMM runs:
  sum(y)   = 1^T W_eff xs        (xs = sum_n x, host-prepared)
  sum(y^2) = tr((W_eff^T W_eff) G) = <M, G>   (G already on chip)
so the per-channel scale/offset are known up front, the normalization is
fused into the y-psum evacuation, and each output tile is DMA'd to HBM
as soon as it is produced: no bn_stats, no apply pass, no output drain.

Sharding: pure data-parallel over batch: 16 samples / 8 cores = 2 per core.

Pipeline (PE queue order; b0/b1 are the two per-core batches):
  G0 T0 sc0 | G1 | R0 W20 M0 | T1 sc1 (stat mms b0) | By0[0:6] |
  R1 W21 M1 | By0[6:8] (stat mms b1) | By1
softmax/stat chains run on DVE under the covering PE phases.
"""

import numpy as np
from contextlib import ExitStack

import concourse.bass as bass
import concourse.tile as tile
from concourse import bacc, mybir
from concourse.bass_utils import run_bass_kernel_spmd
from concourse.masks import make_identity

F32 = mybir.dt.float32
F16 = mybir.dt.float16
I32 = mybir.dt.int32
AX = mybir.AxisListType
ALU = mybir.AluOpType
ACTF = mybir.ActivationFunctionType

B = 16          # global batch
C = 512         # channels
N = 4096        # pixels (64*64)
HW_SIDE = 64
NCORES = 8
PB = B // NCORES  # batches per core
P = 128
KC = C // P     # 4 channel chunks
NB = 8          # n blocks of 512 (y GEMM)
NT = 16         # xT tiles of 2 n-chunks each (G GEMM)
NS = N // 512   # 8 pixel chunks of 512
NHP = 4         # head pairs
XLOOK = 8       # xT DMA lookahead tiles
EPS = 1e-5
CN = C * N


def build_nc():
    nc = bacc.Bacc("TRN2", target_bir_lowering=False, debug=False,
                   num_devices=NCORES)

    # xT[b, t, p, j*512+c] = x[b, c, (2t+j)*128 + p]
    xt_d = nc.declare_dram_parameter("xt", [PB, NT, P, 1024], F16, isOutput=False)
    # x[b, nb, p, k*512+n] = x[b, k*128+p, nb*512+n]
    x_d = nc.declare_dram_parameter("x", [PB, NB, P, KC * 512], F16, isOutput=False)
    wq_d = nc.declare_dram_parameter("wq", [P, KC, C], F16, isOutput=False)   # w_q^T
    wk_d = nc.declare_dram_parameter("wk", [P, KC, C], F16, isOutput=False)   # w_k^T
    wv_d = nc.declare_dram_parameter("wv", [P, KC, C], F16, isOutput=False)   # w_v
    wo_d = nc.declare_dram_parameter("wo", [P, KC, C], F16, isOutput=False)   # w_out^T
    xs_d = nc.declare_dram_parameter("xs", [P, PB, KC], F16, isOutput=False)
    bias_d = nc.declare_dram_parameter("bvec", [P, KC], F32, isOutput=False)
    gamma_d = nc.declare_dram_parameter("gamma", [P, KC], F32, isOutput=False)
    beta_d = nc.declare_dram_parameter("beta", [P, KC], F32, isOutput=False)
    bconst_d = nc.declare_dram_parameter("bconst", [1, 2], F32, isOutput=False)
    out_d = nc.declare_dram_parameter("out", [PB, C, N], F16, isOutput=True)

    with tile.TileContext(nc) as tc, ExitStack() as ctx:
        consts = ctx.enter_context(tc.tile_pool(name="consts", bufs=1))
        xtpool = ctx.enter_context(tc.tile_pool(name="xtpool", bufs=XLOOK + 2))
        xpool = ctx.enter_context(tc.tile_pool(name="xpool", bufs=10))
        gpool = ctx.enter_context(tc.tile_pool(name="gpool", bufs=2))
        tpool = ctx.enter_context(tc.tile_pool(name="tpool", bufs=2))
        rpool = ctx.enter_context(tc.tile_pool(name="rpool", bufs=2))
        w2pool = ctx.enter_context(tc.tile_pool(name="w2pool", bufs=2))
        w2tpool = ctx.enter_context(tc.tile_pool(name="w2tpool", bufs=2))
        mpool = ctx.enter_context(tc.tile_pool(name="mpool", bufs=2))
        ybuf = ctx.enter_context(tc.tile_pool(name="ybuf", bufs=10))
        attn = ctx.enter_context(tc.tile_pool(name="attn", bufs=8))
        attnt = ctx.enter_context(tc.tile_pool(name="attnt", bufs=4))
        stats = ctx.enter_context(tc.tile_pool(name="stats", bufs=4))
        # psg serves G (4 full banks), the scores tiles and the w2-transpose
        # staging: a matmul start=True resets the target bank's whole
        # per-partition row, so concurrent accumulators need separate banks
        # (partition packing 0:64/64:128 within a bank is safe, free-offset
        # packing is NOT).
        psg = ctx.enter_context(tc.tile_pool(name="psg", bufs=4, space="PSUM"))
        psmm = ctx.enter_context(tc.tile_pool(name="psmm", bufs=4, space="PSUM"))

        def load_w(dram):
            t = consts.tile([P, KC, C], F16, tag=f"w_{dram.name}")
            nc.sync.dma_start(out=t, in_=dram[:, :, :])
            return t

        xt_tiles = {}

        def fetch_xt(b, t):
            xt = xtpool.tile([P, 2, 512], F16, tag="xt", name=f"xt_{b}_{t}")
            nc.sync.dma_start(
                out=xt, in_=xt_d[b, t].rearrange("p (j c) -> p j c", j=2))
            xt_tiles[(b, t)] = xt

        # xT tiles first: weight loads are deferred until G0's xT stream is
        # fully issued (weights are only needed from T0 onward), so the
        # first G matmuls are never DMA-supply-gated.
        for t in range(XLOOK):
            fetch_xt(0, t)
        W = {}

        xs_sb = consts.tile([P, PB, KC], F16, tag="xs")
        nc.gpsimd.dma_start(out=xs_sb, in_=xs_d[:, :, :])
        bias_sb = consts.tile([P, KC], F32, tag="bias")
        nc.gpsimd.dma_start(out=bias_sb, in_=bias_d[:, :])
        gamma_sb = consts.tile([P, KC], F32, tag="gamma")
        nc.gpsimd.dma_start(out=gamma_sb, in_=gamma_d[:, :])
        beta_sb = consts.tile([P, KC], F32, tag="beta")
        nc.gpsimd.dma_start(out=beta_sb, in_=beta_d[:, :])
        bconst_sb = consts.tile([1, 2], F32, tag="bconst")
        nc.gpsimd.dma_start(out=bconst_sb, in_=bconst_d[:, :])

        ident_sb = consts.tile([P, P], F16, tag="ident")
        make_identity(nc, ident_sb)
        eps_sb = consts.tile([1, 1], F32, tag="eps")
        nc.vector.memset(eps_sb, EPS)
        # pre-warm the exp activation table (softmax) so no ACT table load
        # lands mid-kernel; the stats chain's rsqrt runs DVE-only.
        warm_sb = consts.tile([1, 1], F32, tag="warm")
        nc.scalar.activation(out=warm_sb, in_=eps_sb, func=ACTF.Exp,
                             bias=0.0, scale=0.0)
        magic_sb = consts.tile([1, 1], I32, tag="magic")
        nc.vector.memset(magic_sb, 0x5f3759df)
        c15_sb = consts.tile([1, 1], F32, tag="c15")
        nc.vector.memset(c15_sb, 1.5)
        ones_col = consts.tile([P, 1], F32, tag="ones_col")
        nc.vector.memset(ones_col, 1.0)
        ones_row = consts.tile([1, P], F32, tag="ones_row")
        nc.vector.memset(ones_row, 1.0)

        # per-batch state carried between emission stages
        st_g = {}    # G in SBUF (f16) [P, KC, C]
        st_t = {}    # T = G @ wk^T   [P, KC, C]
        st_sc = {}   # scores psum tiles
        st_at = {}   # block-diag attn tiles
        st_r = {}    # R = bd(A)^T @ wo^T
        st_w2 = {}   # W_effT = wv^T-contract @ R
        st_ts = {}   # analytic-stats intermediates
        st_scale = {}
        st_by = {}

        def emit_G(b, prefetched, hook=None):
            """G = x x^T, t-outer: each xT tile is consumed then retired.
            Only the upper block-triangle is computed (rhs = cols >= m*128);
            the 6 lower [128,128] blocks are PE-transposed from the upper
            copies.  All 4 chunk accumulators live in 4 psum banks."""
            g_sb = gpool.tile([P, KC, C], F16, tag="g", name=f"g_{b}")
            st_g[b] = g_sb
            ps = [psg.tile([P, C - m * P], F32, tag="psg", name=f"g_{b}_{m}")
                  for m in range(KC)]
            for t in range(NT):
                tf = t + prefetched
                if tf < NT:
                    fetch_xt(b, tf)
                elif b + 1 < PB and tf - NT < NT:
                    fetch_xt(b + 1, tf - NT)
                if hook and t in hook:
                    hook[t]()
                xt = xt_tiles.pop((b, t))
                for j in range(2):
                    for m in range(KC):
                        nc.tensor.matmul(
                            ps[m],
                            lhsT=xt[:, j, m * P:(m + 1) * P],
                            rhs=xt[:, j, m * P:],
                            start=(t == 0 and j == 0),
                            stop=(t == NT - 1 and j == 1),
                            skip_group_check=True)
            for m in range(KC):
                if m % 2 == 0:
                    nc.scalar.copy(out=g_sb[:, m, m * P:], in_=ps[m])
                else:
                    nc.vector.tensor_copy(out=g_sb[:, m, m * P:], in_=ps[m])
            # lower blocks (m, mp<m) = transpose(upper block (mp, m)),
            # ordered so T chunk 0's operands are ready first
            for m, mp in ((1, 0), (2, 0), (3, 0), (2, 1), (3, 1), (3, 2)):
                pst = psmm.tile([P, P], F16, tag="psmm")
                nc.tensor.transpose(
                    pst, g_sb[:, mp, m * P:(m + 1) * P], ident_sb)
                if (m + mp) % 2 == 0:
                    nc.vector.tensor_copy(
                        out=g_sb[:, m, mp * P:(mp + 1) * P], in_=pst)
                else:
                    nc.scalar.copy(
                        out=g_sb[:, m, mp * P:(mp + 1) * P], in_=pst)

        def emit_T(b):
            """T = G @ wk^T  [c, e], m-outer single-bank accumulation."""
            g_sb = st_g[b]
            t_sb = tpool.tile([P, KC, C], F16, tag="t", name=f"t_{b}")
            st_t[b] = t_sb
            for m in range(KC):
                ps = psmm.tile([P, C], F32, tag="psmm")
                for mp in range(KC):
                    nc.tensor.matmul(
                        ps,
                        lhsT=g_sb[:, mp, m * P:(m + 1) * P],
                        rhs=W['wk'][:, mp, :],
                        start=(mp == 0), stop=(mp == KC - 1))
                nc.vector.tensor_copy(out=t_sb[:, m, :], in_=ps)

        def emit_scores(b):
            """scores_h = (w_q T)_h, two heads packed per psum tile; k-outer
            so the first matmuls only need T chunk 0."""
            t_sb = st_t[b]
            sc_ps = [psg.tile([P, 64], F32, tag="psg", name=f"sc_{b}_{hp}")
                     for hp in range(NHP)]
            st_sc[b] = sc_ps
            for k in range(KC):
                for hp in range(NHP):
                    hA, hB = 2 * hp, 2 * hp + 1
                    clA = slice(hA * 64, hA * 64 + 64)
                    clB = slice(hB * 64, hB * 64 + 64)
                    nc.tensor.matmul(
                        sc_ps[hp][0:64, :],
                        lhsT=W['wq'][:, k, clA], rhs=t_sb[:, k, clA],
                        start=(k == 0), stop=(k == KC - 1),
                        skip_group_check=True)
                    nc.tensor.matmul(
                        sc_ps[hp][64:P, :],
                        lhsT=W['wq'][:, k, clB], rhs=t_sb[:, k, clB],
                        start=(k == 0), stop=(k == KC - 1),
                        skip_group_check=True)

        def emit_softmax(b):
            """softmax over scores (all head pairs batched) -> blockdiag tiles."""
            sc_ps = st_sc[b]
            a_all = attn.tile([P, NHP, 64], F32, tag="a_all")
            for hp in range(NHP):
                nc.vector.tensor_copy(out=a_all[:, hp, :], in_=sc_ps[hp])
            mx = attn.tile([P, NHP, 1], F32, tag="mx4")
            nc.vector.reduce_max(out=mx, in_=a_all, axis=AX.X)
            d_all = attn.tile([P, NHP, 64], F32, tag="d_all")
            nc.vector.tensor_tensor(d_all, a_all,
                                    mx.to_broadcast([P, NHP, 64]), ALU.subtract)
            e_all = attn.tile([P, NHP, 64], F32, tag="e_all")
            nc.scalar.activation(out=e_all, in_=d_all, func=ACTF.Exp,
                                 bias=0.0, scale=0.125)
            sm = attn.tile([P, NHP, 1], F32, tag="sm4")
            nc.vector.reduce_sum(out=sm, in_=e_all, axis=AX.X)
            rs = attn.tile([P, NHP, 1], F32, tag="rs4")
            nc.vector.reciprocal(out=rs, in_=sm)
            a_mm = attn.tile([P, NHP, 64], F16, tag="amm4")
            nc.vector.tensor_tensor(a_mm, e_all,
                                    rs.to_broadcast([P, NHP, 64]), ALU.mult)
            bd_tiles = []
            for hp in range(NHP):
                at = attnt.tile([P, P], F16, tag="attnT", name=f"at_{b}_{hp}")
                nc.gpsimd.memset(at, 0.0)
                nc.vector.tensor_copy(out=at[0:64, 0:64], in_=a_mm[0:64, hp, :])
                nc.vector.tensor_copy(out=at[64:P, 64:P], in_=a_mm[64:P, hp, :])
                bd_tiles.append(at)
            st_at[b] = bd_tiles

        def emit_R(b):
            """R[e, o] = sum_d bd(A)[d, e] wo^T[d, o]."""
            bd_tiles = st_at[b]
            r_sb = rpool.tile([P, KC, C], F16, tag="r", name=f"r_{b}")
            st_r[b] = r_sb
            for hp in range(NHP):
                ps = psmm.tile([P, C], F32, tag="psmm")
                nc.tensor.matmul(ps, lhsT=bd_tiles[hp], rhs=W['wo'][:, hp, :],
                                 start=True, stop=True)
                if hp % 2 == 0:
                    nc.scalar.copy(out=r_sb[:, hp, :], in_=ps)
                else:
                    nc.vector.tensor_copy(out=r_sb[:, hp, :], in_=ps)

        def emit_W2(b):
            """W_effT[c, o] = sum_e wv[e, c] R[e, o], m-outer."""
            r_sb = st_r[b]
            w2 = w2pool.tile([P, KC, C], F16, tag="w2", name=f"w2_{b}")
            st_w2[b] = w2
            for m in range(KC):
                ps = psmm.tile([P, C], F32, tag="psmm")
                for ki in range(KC):
                    nc.tensor.matmul(
                        ps,
                        lhsT=W['wv'][:, ki, m * P:(m + 1) * P],
                        rhs=r_sb[:, ki, :],
                        start=(ki == 0), stop=(ki == KC - 1))
                if m % 2 == 0:
                    nc.scalar.copy(out=w2[:, m, :], in_=ps)
                else:
                    nc.vector.tensor_copy(out=w2[:, m, :], in_=ps)

        def emit_M(b):
            """M = W_eff^T W_eff (upper block-triangle) + ws = W_eff @ xs.
            w2 is transposed on the PE (mo-outer so the M accumulation can
            chase the transpose copies), then M accumulates like G."""
            w2 = st_w2[b]
            w2t = w2tpool.tile([P, KC, C], F16, tag="w2t", name=f"w2t_{b}")
            for mo in range(KC):
                for k in range(KC):
                    pst = psg.tile([P, P], F16, tag="psg")
                    nc.tensor.transpose(
                        pst, w2[:, k, mo * P:(mo + 1) * P], ident_sb)
                    if k % 2 == 0:
                        nc.scalar.copy(
                            out=w2t[:, mo, k * P:(k + 1) * P], in_=pst)
                    else:
                        nc.vector.tensor_copy(
                            out=w2t[:, mo, k * P:(k + 1) * P], in_=pst)
            ps = [psmm.tile([P, C - m * P], F32, tag="psmm", name=f"M_{b}_{m}")
                  for m in range(KC)]
            for mo in range(KC):
                for m in range(KC):
                    nc.tensor.matmul(
                        ps[m],
                        lhsT=w2t[:, mo, m * P:(m + 1) * P],
                        rhs=w2t[:, mo, m * P:],
                        start=(mo == 0), stop=(mo == KC - 1),
                        skip_group_check=True)
            st_ts[b] = {"mps": ps}

        def emit_ws(b):
            """ws[o] = sum_c W_eff[o, c] xs[c] (PE, tiny)."""
            w2 = st_w2[b]
            ws_sb = stats.tile([P, KC], F32, tag="ws")
            for m in range(KC):
                wps = psmm.tile([P, 1], F32, tag="psmm")
                for k in range(KC):
                    nc.tensor.matmul(
                        wps,
                        lhsT=w2[:, k, m * P:(m + 1) * P],
                        rhs=xs_sb[:, b, k:k + 1],
                        start=(k == 0), stop=(k == KC - 1))
                nc.vector.tensor_copy(out=ws_sb[:, m:m + 1], in_=wps)
            st_ts[b]["ws"] = ws_sb

        def emit_stats_a(b):
            """DVE: tr(M G) = sum(2*upper - diag) read straight from the
            M psum banks (no SBUF staging) -> trp partials."""
            mps = st_ts[b]["mps"]
            g_sb = st_g[b]
            trp = stats.tile([P, 8], F32, tag="trp")
            scr = stats.tile([P, C], F32, tag="scr")
            for m in range(KC):
                w = C - m * P
                nc.vector.tensor_tensor(
                    scr[:, :w], mps[m], g_sb[:, m, m * P:], ALU.mult)
                nc.vector.reduce_sum(out=trp[:, m:m + 1], in_=scr[:, :w],
                                     axis=AX.X)
                nc.vector.reduce_sum(out=trp[:, KC + m:KC + m + 1],
                                     in_=scr[:, 0:P], axis=AX.X)
            st_ts[b]["trp"] = trp

        def emit_stats_sv(b):
            """sv = [2*sum(upper) - sum(diag), sum(ws), sum(ws*b)]."""
            trp = st_ts[b]["trp"]
            ws = st_ts[b]["ws"]
            sv = stats.tile([P, 3], F32, tag="sv")
            dd = stats.tile([P, 2], F32, tag="dd")
            nc.vector.reduce_sum(out=dd[:, 0:1], in_=trp[:, 0:KC], axis=AX.X)
            nc.vector.reduce_sum(out=dd[:, 1:2], in_=trp[:, KC:2 * KC],
                                 axis=AX.X)
            nc.vector.tensor_scalar(
                out=sv[:, 0:1], in0=dd[:, 0:1], scalar1=2.0, scalar2=None,
                op0=ALU.mult)
            nc.vector.tensor_sub(sv[:, 0:1], sv[:, 0:1], dd[:, 1:2])
            u = stats.tile([P, KC], F32, tag="u")
            nc.vector.tensor_mul(u, ws, bias_sb)
            nc.vector.reduce_sum(out=sv[:, 1:2], in_=ws, axis=AX.X)
            nc.vector.reduce_sum(out=sv[:, 2:3], in_=u, axis=AX.X)
            st_ts[b]["sv"] = sv

        def emit_stats_mm(b):
            """cross-partition reduce of [tr, S1, S2] (PE)."""
            sv = st_ts[b]["sv"]
            p3 = psmm.tile([1, 3], F32, tag="psmm", name=f"p3_{b}")
            nc.tensor.matmul(p3, lhsT=ones_col, rhs=sv, start=True, stop=True)
            st_ts[b]["p3"] = p3

        def emit_stats_b(b):
            """scalar combine: mu, var, rstd (DVE-only quake rsqrt)."""
            p3 = st_ts[b]["p3"]
            sc2 = stats.tile([1, 2], F32, tag="sc2")
            # mu = S1/CN + B1/C
            nc.vector.tensor_scalar(
                out=sc2[:, 0:1], in0=p3[:, 1:2], scalar1=1.0 / CN,
                scalar2=None, op0=ALU.mult)
            nc.vector.tensor_add(sc2[:, 0:1], sc2[:, 0:1], bconst_sb[:, 0:1])
            # E2 = TR/CN + 2*S2/CN + B2/C ; var = E2 - mu^2 + eps
            vb = stats.tile([1, 1], F32, tag="vb")
            nc.vector.tensor_scalar(
                out=vb, in0=p3[:, 2:3], scalar1=2.0 / CN, scalar2=None,
                op0=ALU.mult)
            t2 = stats.tile([1, 1], F32, tag="t2")
            nc.vector.tensor_scalar(
                out=t2, in0=p3[:, 0:1], scalar1=1.0 / CN, scalar2=None,
                op0=ALU.mult)
            nc.vector.tensor_add(vb, vb, t2)
            nc.vector.tensor_add(vb, vb, bconst_sb[:, 1:2])
            m2 = stats.tile([1, 1], F32, tag="m2")
            nc.vector.tensor_mul(m2, sc2[:, 0:1], sc2[:, 0:1])
            nc.vector.tensor_sub(vb, vb, m2)
            nc.vector.tensor_add(vb, vb, eps_sb)
            # rstd = 1/sqrt(vb): quake bit trick + 2 Newton steps (DVE only)
            hv = stats.tile([1, 1], F32, tag="hv")
            nc.vector.tensor_scalar_mul(hv, vb, 0.5)
            r = stats.tile([1, 1], F32, tag="rq")
            nc.vector.tensor_scalar(
                out=r.bitcast(I32), in0=vb.bitcast(I32),
                scalar1=1, scalar2=None, op0=ALU.logical_shift_right)
            nc.vector.tensor_tensor(r.bitcast(I32), magic_sb,
                                    r.bitcast(I32), ALU.subtract)
            t1 = stats.tile([1, 1], F32, tag="t1")
            for _ in range(2):
                nc.vector.tensor_mul(t1, r, r)
                nc.vector.tensor_mul(t1, hv, t1)
                nc.vector.tensor_sub(t1, c15_sb, t1)
                nc.vector.tensor_mul(r, r, t1)
            nc.vector.tensor_copy(out=sc2[:, 1:2], in_=r)
            st_ts[b]["sc2"] = sc2

        def emit_stats_bc(b):
            """broadcast mu/rstd to all partitions (PE)."""
            sc2 = st_ts[b]["sc2"]
            bc_ps = psmm.tile([P, 2], F32, tag="psmm", name=f"bc_{b}")
            nc.tensor.matmul(bc_ps, lhsT=ones_row, rhs=sc2,
                             start=True, stop=True)
            st_ts[b]["bc"] = bc_ps

        def emit_stats_c(b):
            """per-channel scale/offset: s = gamma*rstd,
            t = beta + (bias - mu)*s."""
            bc_ps = st_ts[b]["bc"]
            s_ch = stats.tile([P, KC], F32, tag="s_ch")
            nc.vector.tensor_scalar_mul(s_ch, gamma_sb, bc_ps[:, 1:2])
            u = stats.tile([P, KC], F32, tag="tb")
            nc.vector.tensor_scalar(
                out=u, in0=bias_sb, scalar1=bc_ps[:, 0:1], scalar2=None,
                op0=ALU.subtract)
            t_ch = stats.tile([P, KC], F32, tag="t_ch")
            nc.vector.tensor_mul(t_ch, u, s_ch)
            nc.vector.tensor_add(t_ch, t_ch, beta_sb)
            st_scale[b] = (s_ch, t_ch)

        def emit_By_setup(b):
            x_blks = {}
            for ns in range(NS):
                xb = xpool.tile([P, KC, 512], F16, tag="xblk",
                                name=f"x_{b}_{ns}")
                nc.sync.dma_start(
                    out=xb, in_=x_d[b, ns].rearrange("p (k n) -> p k n", k=KC))
                x_blks[ns] = xb
            st_by[b] = (x_blks, {})

        def emit_By_blocks(b, blocks):
            """y = W_eff @ x with the groupnorm apply FUSED into the psum
            evacuation (s/t known up front); finished [P,1024] output tiles
            are DMA'd to HBM immediately."""
            w2 = st_w2[b]
            s_ch, t_ch = st_scale[b]
            x_blks, ybufs = st_by[b]
            for ns in blocks:
                pair, half = ns // 2, ns % 2
                x_blk = x_blks.pop(ns)
                for m in range(KC):
                    if half == 0:
                        yb = ybuf.tile([P, 1024], F16, tag="yb",
                                       name=f"yb_{b}_{m}_{pair}")
                        ybufs[(m, pair)] = yb
                    else:
                        yb = ybufs[(m, pair)]
                    ps = psmm.tile([P, 512], F32, tag="psmm")
                    for k in range(KC):
                        nc.tensor.matmul(
                            ps,
                            lhsT=w2[:, k, m * P:(m + 1) * P],
                            rhs=x_blk[:, k, :],
                            start=(k == 0), stop=(k == KC - 1))
                    ysl = yb[:, half * 512:(half + 1) * 512]
                    if m % 2 == 1:
                        nc.vector.tensor_scalar(
                            out=ysl, in0=ps,
                            scalar1=s_ch[:, m:m + 1], scalar2=t_ch[:, m:m + 1],
                            op0=ALU.mult, op1=ALU.add)
                    else:
                        nc.scalar.activation(
                            out=ysl, in_=ps, func=ACTF.Identity,
                            bias=t_ch[:, m:m + 1], scale=s_ch[:, m:m + 1])
                    if half == 1:
                        nc.sync.dma_start(
                            out=out_d[b, m * P:(m + 1) * P,
                                      pair * 1024:(pair + 1) * 1024],
                            in_=yb)

        # ---- emission schedule (PE queue order is emission order) ----
        emit_G(0, prefetched=XLOOK, hook={
            NT - 1 - XLOOK: lambda: W.update(wk=load_w(wk_d),
                                             wq=load_w(wq_d))})
        W.update(wo=load_w(wo_d), wv=load_w(wv_d))
        emit_T(0)
        emit_scores(0)
        emit_softmax(0)       # DVE, overlaps G1 on PE
        emit_G(1, prefetched=XLOOK)
        emit_R(0)
        emit_W2(0)
        emit_M(0)
        emit_stats_a(0)       # DVE reads M psum, overlaps T1 on PE
        emit_T(1)
        emit_ws(0)
        emit_stats_sv(0)
        emit_scores(1)
        emit_stats_mm(0)
        emit_stats_b(0)       # DVE, overlaps sc1/softmax1 queueing
        emit_softmax(1)
        emit_stats_bc(0)
        emit_stats_c(0)
        emit_By_setup(0)
        emit_By_blocks(0, range(6))
        emit_R(1)
        emit_W2(1)
        emit_M(1)
        emit_stats_a(1)       # DVE, overlaps By0 tail on PE
        emit_By_blocks(0, range(6, 7))
        emit_ws(1)
        emit_stats_sv(1)
        emit_stats_mm(1)
        emit_stats_b(1)
        emit_By_blocks(0, range(7, NS))
        emit_stats_bc(1)
        emit_stats_c(1)
        emit_By_setup(1)
        emit_By_blocks(1, range(NS))

    nc.finalize()
    return nc


_NC_CACHE = {}


def _get_nc():
    if "nc" not in _NC_CACHE:
        _NC_CACHE["nc"] = build_nc()
    return _NC_CACHE["nc"]


def _prep_w(w):
    # [C_in, C_out] -> [128, KC, C_out] fp16 with c_in = k*128 + p
    return np.ascontiguousarray(
        w.reshape(KC, P, C).transpose(1, 0, 2).astype(np.float16))


def _prep_vec(v):
    # [C] -> [128, KC] with c = k*128 + p
    return np.ascontiguousarray(v.reshape(KC, P).T)


def _prep_x(x):
    # [B, C, N] -> [B, NB, P, KC*512] fp16: block j, partition p, (k, n)
    nb = x.shape[0]
    xr = x.reshape(nb, KC, P, NB, 512)
    return np.ascontiguousarray(
        xr.transpose(0, 3, 2, 1, 4).astype(np.float16)).reshape(
        nb, NB, P, KC * 512)


def _prep_xt(x):
    # [B, C, N] -> [B, NT, P, 2*512] fp16: xt[b,t,p,j*512+c] = x[b,c,(2t+j)*128+p]
    nb = x.shape[0]
    xr = x.reshape(nb, C, NT, 2, P)           # [b, c, t, j, p]
    return np.ascontiguousarray(
        xr.transpose(0, 2, 4, 3, 1).astype(np.float16)).reshape(
        nb, NT, P, 1024)


def _make_in_maps(x, w_qkv, w_out, b_out, gamma, beta):
    x = np.asarray(x, dtype=np.float32).reshape(B, C, N)
    xr = _prep_x(x)
    xtr = _prep_xt(x)
    # xs[p, b, k] = sum_n x[b, k*128+p, n]
    xs = x.sum(axis=2).reshape(B, KC, P).transpose(2, 0, 1)
    xs = np.ascontiguousarray(xs.astype(np.float16))
    w_qkv = np.asarray(w_qkv, dtype=np.float32)
    wq = _prep_w(np.ascontiguousarray(w_qkv[0:C].T))
    wk = _prep_w(np.ascontiguousarray(w_qkv[C:2 * C].T))
    wv = _prep_w(np.ascontiguousarray(w_qkv[2 * C:3 * C]))
    wo = _prep_w(np.ascontiguousarray(np.asarray(w_out, dtype=np.float32).T))
    b_out = np.asarray(b_out, dtype=np.float32)
    bvec = _prep_vec(b_out)
    gam = _prep_vec(np.asarray(gamma, dtype=np.float32))
    bet = _prep_vec(np.asarray(beta, dtype=np.float32))
    bconst = np.array([[b_out.sum() / C, (b_out * b_out).sum() / C]],
                      dtype=np.float32)
    return [
        dict(x=np.ascontiguousarray(xr[c * PB:(c + 1) * PB]),
             xt=np.ascontiguousarray(xtr[c * PB:(c + 1) * PB]),
             xs=np.ascontiguousarray(xs[:, c * PB:(c + 1) * PB]),
             wq=wq, wk=wk, wv=wv, wo=wo,
             bvec=bvec, gamma=gam, beta=bet, bconst=bconst)
        for c in range(NCORES)
    ]


def _run(inputs, trace=False, trace_kwargs=None):
    nc = _get_nc()
    in_maps = _make_in_maps(**inputs)
    res = run_bass_kernel_spmd(nc, in_maps, core_ids=list(range(NCORES)),
                               trace=trace, **(trace_kwargs or {}))
    out = np.concatenate([res.results[c]["out"].astype(np.float32)
                          for c in range(NCORES)], axis=0)
    return out.reshape(B, C, HW_SIDE, HW_SIDE), res


def kernel(x, w_qkv, w_out, b_out, gamma, beta):
    inputs = dict(x=x, w_qkv=w_qkv, w_out=w_out, b_out=b_out,
                  gamma=gamma, beta=beta)
    try:
        out, _ = _run(inputs)
    except Exception:
        # transient device errors (e.g. NRT_EXEC_UNIT_UNRECOVERABLE) have
        # been observed once across many runs; one retry recovers.
        out, _ = _run(inputs)
    return out


# revision 43
# speedup vs baseline: 1.0753x; 1.0218x over previous
"""Trainium2 Bass kernel for nn_MultiHeadAttention_63814624084186.

Reference computation (per batch sample b, fully independent across b):
  x: [512, 4096]  (C channels x N=64*64 pixels)
  qkv = w_qkv @ x            -> q,k,v each [512, 4096] (8 heads x 64 dims)
  scores = (q_h @ k_h^T)/8   -> [64, 64] per head   (channel-attention)
  attn = softmax(scores, -1)
  out_h = attn_h @ v_h       -> [64, 4096]
  y = w_out @ out + b_out    -> [512, 4096]
  y = groupnorm(y over all C,N) * gamma + beta

Key algebra (this version): attention is over the CHANNEL dim, so
  scores_h = q_h k_h^T = (w_q G w_k^T)_h   with  G = x x^T  [512,512]
  y = w_out bd(A) w_v x = W_eff x          with  W_eff folded on-chip
q, k, v are never materialized.  Per-batch PE work drops from ~4.5e9
MACs (qkv + v + out-proj) to ~2.4e9 (G + y GEMM + small folds).

GroupNorm stats are computed ANALYTICALLY before the y GEMM runs:
  sum(y)   = 1^T W_eff xs        (xs = sum_n x, host-prepared)
  sum(y^2) = tr((W_eff^T W_eff) G) = <M, G>   (G already on chip)
so the per-channel scale/offset are known up front, the normalization is
fused into the y-psum evacuation, and each output tile is DMA'd to HBM
as soon as it is produced: no bn_stats, no apply pass, no output drain.

Sharding: pure data-parallel over batch: 16 samples / 8 cores = 2 per core.

Pipeline (PE queue order; b0/b1 are the two per-core batches):
  G0 T0 sc0 | G1 | R0 W20 M0 | T1 sc1 (stat mms b0) | By0[0:6] |
  R1 W21 M1 | By0[6:8] (stat mms b1) | By1
softmax/stat chains run on DVE under the covering PE phases.
"""

import numpy as np
from contextlib import ExitStack

import concourse.bass as bass
import concourse.tile as tile
from concourse import bacc, mybir
from concourse.bass_utils import run_bass_kernel_spmd
from concourse.masks import make_identity

F32 = mybir.dt.float32
F16 = mybir.dt.float16
I32 = mybir.dt.int32
AX = mybir.AxisListType
ALU = mybir.AluOpType
ACTF = mybir.ActivationFunctionType

B = 16          # global batch
C = 512         # channels
N = 4096        # pixels (64*64)
HW_SIDE = 64
NCORES = 8
PB = B // NCORES  # batches per core
P = 128
KC = C // P     # 4 channel chunks
NB = 8          # n blocks of 512 (y GEMM)
NT = 16         # xT tiles of 2 n-chunks each (G GEMM)
NS = N // 512   # 8 pixel chunks of 512
NHP = 4         # head pairs
XLOOK = 8       # xT DMA lookahead tiles
EPS = 1e-5
CN = C * N


def build_nc():
    nc = bacc.Bacc("TRN2", target_bir_lowering=False, debug=False,
                   num_devices=NCORES)

    # xT[b, t, p, j*512+c] = x[b, c, (2t+j)*128 + p]
    xt_d = nc.declare_dram_parameter("xt", [PB, NT, P, 1024], F16, isOutput=False)
    # x[b, nb, p, k*512+n] = x[b, k*128+p, nb*512+n]
    x_d = nc.declare_dram_parameter("x", [PB, NB, P, KC * 512], F16, isOutput=False)
    wq_d = nc.declare_dram_parameter("wq", [P, KC, C], F16, isOutput=False)   # w_q^T
    wk_d = nc.declare_dram_parameter("wk", [P, KC, C], F16, isOutput=False)   # w_k^T
    wv_d = nc.declare_dram_parameter("wv", [P, KC, C], F16, isOutput=False)   # w_v
    wo_d = nc.declare_dram_parameter("wo", [P, KC, C], F16, isOutput=False)   # w_out^T
    xs_d = nc.declare_dram_parameter("xs", [P, PB, KC], F16, isOutput=False)
    bias_d = nc.declare_dram_parameter("bvec", [P, KC], F32, isOutput=False)
    gamma_d = nc.declare_dram_parameter("gamma", [P, KC], F32, isOutput=False)
    beta_d = nc.declare_dram_parameter("beta", [P, KC], F32, isOutput=False)
    bconst_d = nc.declare_dram_parameter("bconst", [1, 2], F32, isOutput=False)
    out_d = nc.declare_dram_parameter("out", [PB, C, N], F16, isOutput=True)

    with tile.TileContext(nc) as tc, ExitStack() as ctx:
        consts = ctx.enter_context(tc.tile_pool(name="consts", bufs=1))
        xtpool = ctx.enter_context(tc.tile_pool(name="xtpool", bufs=XLOOK + 2))
        xpool = ctx.enter_context(tc.tile_pool(name="xpool", bufs=10))
        gpool = ctx.enter_context(tc.tile_pool(name="gpool", bufs=2))
        tpool = ctx.enter_context(tc.tile_pool(name="tpool", bufs=2))
        rpool = ctx.enter_context(tc.tile_pool(name="rpool", bufs=2))
        w2pool = ctx.enter_context(tc.tile_pool(name="w2pool", bufs=2))
        w2tpool = ctx.enter_context(tc.tile_pool(name="w2tpool", bufs=2))
        mpool = ctx.enter_context(tc.tile_pool(name="mpool", bufs=2))
        ybuf = ctx.enter_context(tc.tile_pool(name="ybuf", bufs=10))
        attn = ctx.enter_context(tc.tile_pool(name="attn", bufs=8))
        attnt = ctx.enter_context(tc.tile_pool(name="attnt", bufs=4))
        stats = ctx.enter_context(tc.tile_pool(name="stats", bufs=4))
        # psg serves G (4 full banks), the scores tiles and the w2-transpose
        # staging: a matmul start=True resets the target bank's whole
        # per-partition row, so concurrent accumulators need separate banks
        # (partition packing 0:64/64:128 within a bank is safe, free-offset
        # packing is NOT).
        psg = ctx.enter_context(tc.tile_pool(name="psg", bufs=4, space="PSUM"))
        psmm = ctx.enter_context(tc.tile_pool(name="psmm", bufs=4, space="PSUM"))

        def load_w(dram):
            t = consts.tile([P, KC, C], F16, tag=f"w_{dram.name}")
            nc.sync.dma_start(out=t, in_=dram[:, :, :])
            return t

        xt_tiles = {}

        def fetch_xt(b, t):
            xt = xtpool.tile([P, 2, 512], F16, tag="xt", name=f"xt_{b}_{t}")
            nc.sync.dma_start(
                out=xt, in_=xt_d[b, t].rearrange("p (j c) -> p j c", j=2))
            xt_tiles[(b, t)] = xt

        # xT tiles first: weight loads are deferred until G0's xT stream is
        # fully issued (weights are only needed from T0 onward), so the
        # first G matmuls are never DMA-supply-gated.
        for t in range(XLOOK):
            fetch_xt(0, t)
        W = {}

        xs_sb = consts.tile([P, PB, KC], F16, tag="xs")
        nc.gpsimd.dma_start(out=xs_sb, in_=xs_d[:, :, :])
        bias_sb = consts.tile([P, KC], F32, tag="bias")
        nc.gpsimd.dma_start(out=bias_sb, in_=bias_d[:, :])
        gamma_sb = consts.tile([P, KC], F32, tag="gamma")
        nc.gpsimd.dma_start(out=gamma_sb, in_=gamma_d[:, :])
        beta_sb = consts.tile([P, KC], F32, tag="beta")
        nc.gpsimd.dma_start(out=beta_sb, in_=beta_d[:, :])
        bconst_sb = consts.tile([1, 2], F32, tag="bconst")
        nc.gpsimd.dma_start(out=bconst_sb, in_=bconst_d[:, :])

        ident_sb = consts.tile([P, P], F16, tag="ident")
        make_identity(nc, ident_sb)
        eps_sb = consts.tile([1, 1], F32, tag="eps")
        nc.vector.memset(eps_sb, EPS)
        # pre-warm the exp activation table (softmax) so no ACT table load
        # lands mid-kernel; the stats chain's rsqrt runs DVE-only.
        warm_sb = consts.tile([1, 1], F32, tag="warm")
        nc.scalar.activation(out=warm_sb, in_=eps_sb, func=ACTF.Exp,
                             bias=0.0, scale=0.0)
        magic_sb = consts.tile([1, 1], I32, tag="magic")
        nc.vector.memset(magic_sb, 0x5f3759df)
        c15_sb = consts.tile([1, 1], F32, tag="c15")
        nc.vector.memset(c15_sb, 1.5)
        ones_col = consts.tile([P, 1], F32, tag="ones_col")
        nc.vector.memset(ones_col, 1.0)
        ones_row = consts.tile([1, P], F32, tag="ones_row")
        nc.vector.memset(ones_row, 1.0)

        # per-batch state carried between emission stages
        st_g = {}    # G in SBUF (f16) [P, KC, C]
        st_t = {}    # T = G @ wk^T   [P, KC, C]
        st_sc = {}   # scores psum tiles
        st_at = {}   # block-diag attn tiles
        st_r = {}    # R = bd(A)^T @ wo^T
        st_w2 = {}   # W_effT = wv^T-contract @ R
        st_ts = {}   # analytic-stats intermediates
        st_scale = {}
        st_by = {}

        st_gps = {}

        def emit_G(b, prefetched, hook=None, t_range=None):
            """G = x x^T, t-outer: each xT tile is consumed then retired.
            Only the upper block-triangle is computed (rhs = cols >= m*128);
            the 6 lower [128,128] blocks are PE-transposed from the upper
            copies.  All 4 chunk accumulators live in 4 psum banks.
            t_range allows splitting the accumulation into several emission
            segments (psum groups stay open in between)."""
            if t_range is None:
                t_range = range(NT)
            if t_range[0] == 0:
                g_sb = gpool.tile([P, KC, C], F16, tag="g", name=f"g_{b}")
                st_g[b] = g_sb
                st_gps[b] = [psg.tile([P, C - m * P], F32, tag="psg",
                                      name=f"g_{b}_{m}") for m in range(KC)]
            g_sb = st_g[b]
            ps = st_gps[b]
            for t in t_range:
                tf = t + prefetched
                if tf < NT:
                    fetch_xt(b, tf)
                elif b + 1 < PB and tf - NT < NT:
                    fetch_xt(b + 1, tf - NT)
                if hook and t in hook:
                    hook[t]()
                xt = xt_tiles.pop((b, t))
                for j in range(2):
                    for m in range(KC):
                        nc.tensor.matmul(
                            ps[m],
                            lhsT=xt[:, j, m * P:(m + 1) * P],
                            rhs=xt[:, j, m * P:],
                            start=(t == 0 and j == 0),
                            stop=(t == NT - 1 and j == 1),
                            skip_group_check=True)
            if t_range[-1] != NT - 1:
                return
            for m in range(KC):
                if m % 2 == 0:
                    nc.scalar.copy(out=g_sb[:, m, m * P:], in_=ps[m])
                else:
                    nc.vector.tensor_copy(out=g_sb[:, m, m * P:], in_=ps[m])
            # lower blocks (m, mp<m) = transpose(upper block (mp, m)),
            # ordered so T chunk 0's operands are ready first
            for m, mp in ((1, 0), (2, 0), (3, 0), (2, 1), (3, 1), (3, 2)):
                pst = psmm.tile([P, P], F16, tag="psmm")
                nc.tensor.transpose(
                    pst, g_sb[:, mp, m * P:(m + 1) * P], ident_sb)
                if (m + mp) % 2 == 0:
                    nc.vector.tensor_copy(
                        out=g_sb[:, m, mp * P:(mp + 1) * P], in_=pst)
                else:
                    nc.scalar.copy(
                        out=g_sb[:, m, mp * P:(mp + 1) * P], in_=pst)

        def emit_T(b):
            """T = G @ wk^T  [c, e], m-outer single-bank accumulation."""
            g_sb = st_g[b]
            t_sb = tpool.tile([P, KC, C], F16, tag="t", name=f"t_{b}")
            st_t[b] = t_sb
            for m in range(KC):
                ps = psmm.tile([P, C], F32, tag="psmm")
                for mp in range(KC):
                    nc.tensor.matmul(
                        ps,
                        lhsT=g_sb[:, mp, m * P:(m + 1) * P],
                        rhs=W['wk'][:, mp, :],
                        start=(mp == 0), stop=(mp == KC - 1))
                nc.vector.tensor_copy(out=t_sb[:, m, :], in_=ps)

        def emit_scores(b):
            """scores_h = (w_q T)_h, two heads packed per psum tile; k-outer
            so the first matmuls only need T chunk 0."""
            t_sb = st_t[b]
            sc_ps = [psg.tile([P, 64], F32, tag="psg", name=f"sc_{b}_{hp}")
                     for hp in range(NHP)]
            st_sc[b] = sc_ps
            for k in range(KC):
                for hp in range(NHP):
                    hA, hB = 2 * hp, 2 * hp + 1
                    clA = slice(hA * 64, hA * 64 + 64)
                    clB = slice(hB * 64, hB * 64 + 64)
                    nc.tensor.matmul(
                        sc_ps[hp][0:64, :],
                        lhsT=W['wq'][:, k, clA], rhs=t_sb[:, k, clA],
                        start=(k == 0), stop=(k == KC - 1),
                        skip_group_check=True)
                    nc.tensor.matmul(
                        sc_ps[hp][64:P, :],
                        lhsT=W['wq'][:, k, clB], rhs=t_sb[:, k, clB],
                        start=(k == 0), stop=(k == KC - 1),
                        skip_group_check=True)

        def emit_softmax(b):
            """softmax over scores (all head pairs batched) -> blockdiag tiles."""
            sc_ps = st_sc[b]
            a_all = attn.tile([P, NHP, 64], F32, tag="a_all")
            for hp in range(NHP):
                nc.vector.tensor_copy(out=a_all[:, hp, :], in_=sc_ps[hp])
            mx = attn.tile([P, NHP, 1], F32, tag="mx4")
            nc.vector.reduce_max(out=mx, in_=a_all, axis=AX.X)
            d_all = attn.tile([P, NHP, 64], F32, tag="d_all")
            nc.vector.tensor_tensor(d_all, a_all,
                                    mx.to_broadcast([P, NHP, 64]), ALU.subtract)
            e_all = attn.tile([P, NHP, 64], F32, tag="e_all")
            nc.scalar.activation(out=e_all, in_=d_all, func=ACTF.Exp,
                                 bias=0.0, scale=0.125)
            sm = attn.tile([P, NHP, 1], F32, tag="sm4")
            nc.vector.reduce_sum(out=sm, in_=e_all, axis=AX.X)
            rs = attn.tile([P, NHP, 1], F32, tag="rs4")
            nc.vector.reciprocal(out=rs, in_=sm)
            a_mm = attn.tile([P, NHP, 64], F16, tag="amm4")
            nc.vector.tensor_tensor(a_mm, e_all,
                                    rs.to_broadcast([P, NHP, 64]), ALU.mult)
            bd_tiles = []
            for hp in range(NHP):
                at = attnt.tile([P, P], F16, tag="attnT", name=f"at_{b}_{hp}")
                nc.gpsimd.memset(at, 0.0)
                nc.vector.tensor_copy(out=at[0:64, 0:64], in_=a_mm[0:64, hp, :])
                nc.vector.tensor_copy(out=at[64:P, 64:P], in_=a_mm[64:P, hp, :])
                bd_tiles.append(at)
            st_at[b] = bd_tiles

        def emit_R(b):
            """R[e, o] = sum_d bd(A)[d, e] wo^T[d, o]."""
            bd_tiles = st_at[b]
            r_sb = rpool.tile([P, KC, C], F16, tag="r", name=f"r_{b}")
            st_r[b] = r_sb
            for hp in range(NHP):
                ps = psmm.tile([P, C], F32, tag="psmm")
                nc.tensor.matmul(ps, lhsT=bd_tiles[hp], rhs=W['wo'][:, hp, :],
                                 start=True, stop=True)
                if hp % 2 == 0:
                    nc.scalar.copy(out=r_sb[:, hp, :], in_=ps)
                else:
                    nc.vector.tensor_copy(out=r_sb[:, hp, :], in_=ps)

        def emit_W2(b):
            """W_effT[c, o] = sum_e wv[e, c] R[e, o], m-outer."""
            r_sb = st_r[b]
            w2 = w2pool.tile([P, KC, C], F16, tag="w2", name=f"w2_{b}")
            st_w2[b] = w2
            for m in range(KC):
                ps = psmm.tile([P, C], F32, tag="psmm")
                for ki in range(KC):
                    nc.tensor.matmul(
                        ps,
                        lhsT=W['wv'][:, ki, m * P:(m + 1) * P],
                        rhs=r_sb[:, ki, :],
                        start=(ki == 0), stop=(ki == KC - 1))
                if m % 2 == 0:
                    nc.scalar.copy(out=w2[:, m, :], in_=ps)
                else:
                    nc.vector.tensor_copy(out=w2[:, m, :], in_=ps)

        def emit_M(b):
            """M = W_eff^T W_eff (upper block-triangle) + ws = W_eff @ xs.
            w2 is transposed on the PE (mo-outer so the M accumulation can
            chase the transpose copies), then M accumulates like G."""
            w2 = st_w2[b]
            w2t = w2tpool.tile([P, KC, C], F16, tag="w2t", name=f"w2t_{b}")
            # transposes go through the psmm rotation (psg may hold an open
            # G accumulation while this runs); all 16 land before the M mms
            for mo in range(KC):
                for k in range(KC):
                    pst = psmm.tile([P, P], F16, tag="psmm")
                    nc.tensor.transpose(
                        pst, w2[:, k, mo * P:(mo + 1) * P], ident_sb)
                    if k % 2 == 0:
                        nc.scalar.copy(
                            out=w2t[:, mo, k * P:(k + 1) * P], in_=pst)
                    else:
                        nc.vector.tensor_copy(
                            out=w2t[:, mo, k * P:(k + 1) * P], in_=pst)
            ps = [psmm.tile([P, C - m * P], F32, tag="psmm", name=f"M_{b}_{m}")
                  for m in range(KC)]
            for mo in range(KC):
                for m in range(KC):
                    nc.tensor.matmul(
                        ps[m],
                        lhsT=w2t[:, mo, m * P:(m + 1) * P],
                        rhs=w2t[:, mo, m * P:],
                        start=(mo == 0), stop=(mo == KC - 1),
                        skip_group_check=True)
            st_ts[b] = {"mps": ps}

        def emit_ws(b):
            """ws[o] = sum_c W_eff[o, c] xs[c] (PE, tiny)."""
            w2 = st_w2[b]
            ws_sb = stats.tile([P, KC], F32, tag="ws")
            for m in range(KC):
                wps = psmm.tile([P, 1], F32, tag="psmm")
                for k in range(KC):
                    nc.tensor.matmul(
                        wps,
                        lhsT=w2[:, k, m * P:(m + 1) * P],
                        rhs=xs_sb[:, b, k:k + 1],
                        start=(k == 0), stop=(k == KC - 1))
                nc.vector.tensor_copy(out=ws_sb[:, m:m + 1], in_=wps)
            st_ts[b]["ws"] = ws_sb

        def emit_stats_a(b):
            """DVE: tr(M G) = sum(2*upper - diag) read straight from the
            M psum banks (no SBUF staging) -> trp partials."""
            mps = st_ts[b]["mps"]
            g_sb = st_g[b]
            trp = stats.tile([P, 8], F32, tag="trp")
            scr = stats.tile([P, C], F32, tag="scr")
            for m in range(KC):
                w = C - m * P
                nc.vector.tensor_tensor(
                    scr[:, :w], mps[m], g_sb[:, m, m * P:], ALU.mult)
                nc.vector.reduce_sum(out=trp[:, m:m + 1], in_=scr[:, :w],
                                     axis=AX.X)
                nc.vector.reduce_sum(out=trp[:, KC + m:KC + m + 1],
                                     in_=scr[:, 0:P], axis=AX.X)
            st_ts[b]["trp"] = trp

        def emit_stats_sv(b):
            """sv = [2*sum(upper) - sum(diag), sum(ws), sum(ws*b)]."""
            trp = st_ts[b]["trp"]
            ws = st_ts[b]["ws"]
            sv = stats.tile([P, 3], F32, tag="sv")
            dd = stats.tile([P, 2], F32, tag="dd")
            nc.vector.reduce_sum(out=dd[:, 0:1], in_=trp[:, 0:KC], axis=AX.X)
            nc.vector.reduce_sum(out=dd[:, 1:2], in_=trp[:, KC:2 * KC],
                                 axis=AX.X)
            nc.vector.tensor_scalar(
                out=sv[:, 0:1], in0=dd[:, 0:1], scalar1=2.0, scalar2=None,
                op0=ALU.mult)
            nc.vector.tensor_sub(sv[:, 0:1], sv[:, 0:1], dd[:, 1:2])
            u = stats.tile([P, KC], F32, tag="u")
            nc.vector.tensor_mul(u, ws, bias_sb)
            nc.vector.reduce_sum(out=sv[:, 1:2], in_=ws, axis=AX.X)
            nc.vector.reduce_sum(out=sv[:, 2:3], in_=u, axis=AX.X)
            # sv[0] += 2*S2_partial so p3[0] = TR + 2*S2 in one reduce
            nc.vector.tensor_scalar(
                out=sv[:, 2:3], in0=sv[:, 2:3], scalar1=2.0, scalar2=None,
                op0=ALU.mult)
            nc.vector.tensor_add(sv[:, 0:1], sv[:, 0:1], sv[:, 2:3])
            st_ts[b]["sv"] = sv

        def emit_stats_mm(b):
            """cross-partition reduce of [tr, S1, S2] (PE)."""
            sv = st_ts[b]["sv"]
            p3 = psmm.tile([1, 3], F32, tag="psmm", name=f"p3_{b}")
            nc.tensor.matmul(p3, lhsT=ones_col, rhs=sv, start=True, stop=True)
            st_ts[b]["p3"] = p3

        def emit_stats_b(b):
            """scalar combine: mu, var, rstd (DVE-only quake rsqrt)."""
            p3 = st_ts[b]["p3"]
            sc2 = stats.tile([1, 2], F32, tag="sc2")
            # mu = S1/CN + B1/C (fused)
            nc.vector.tensor_scalar(
                out=sc2[:, 0:1], in0=p3[:, 1:2], scalar1=1.0 / CN,
                scalar2=bconst_sb[:, 0:1], op0=ALU.mult, op1=ALU.add)
            # var+eps = (TR + 2*S2)/CN + (B2/C + eps) - mu^2
            vb = stats.tile([1, 1], F32, tag="vb")
            nc.vector.tensor_scalar(
                out=vb, in0=p3[:, 0:1], scalar1=1.0 / CN,
                scalar2=bconst_sb[:, 1:2], op0=ALU.mult, op1=ALU.add)
            m2 = stats.tile([1, 1], F32, tag="m2")
            nc.vector.tensor_mul(m2, sc2[:, 0:1], sc2[:, 0:1])
            nc.vector.tensor_sub(vb, vb, m2)
            # rstd = 1/sqrt(vb): quake bit trick + 1 Newton step (DVE only)
            hv = stats.tile([1, 1], F32, tag="hv")
            nc.vector.tensor_scalar_mul(hv, vb, 0.5)
            r = stats.tile([1, 1], F32, tag="rq")
            nc.vector.tensor_scalar(
                out=r.bitcast(I32), in0=vb.bitcast(I32),
                scalar1=1, scalar2=None, op0=ALU.logical_shift_right)
            nc.vector.tensor_tensor(r.bitcast(I32), magic_sb,
                                    r.bitcast(I32), ALU.subtract)
            t1 = stats.tile([1, 1], F32, tag="t1")
            for _ in range(2):
                nc.vector.tensor_mul(t1, r, r)
                nc.vector.tensor_mul(t1, hv, t1)
                nc.vector.tensor_sub(t1, c15_sb, t1)
                nc.vector.tensor_mul(r, r, t1)
            nc.vector.tensor_copy(out=sc2[:, 1:2], in_=r)
            st_ts[b]["sc2"] = sc2

        def emit_stats_bc(b):
            """broadcast mu/rstd to all partitions (PE)."""
            sc2 = st_ts[b]["sc2"]
            bc_ps = psmm.tile([P, 2], F32, tag="psmm", name=f"bc_{b}")
            nc.tensor.matmul(bc_ps, lhsT=ones_row, rhs=sc2,
                             start=True, stop=True)
            st_ts[b]["bc"] = bc_ps

        def emit_stats_c(b):
            """per-channel scale/offset: s = gamma*rstd,
            t = beta + (bias - mu)*s."""
            bc_ps = st_ts[b]["bc"]
            s_ch = stats.tile([P, KC], F32, tag="s_ch")
            nc.vector.tensor_scalar_mul(s_ch, gamma_sb, bc_ps[:, 1:2])
            u = stats.tile([P, KC], F32, tag="tb")
            nc.vector.tensor_scalar(
                out=u, in0=bias_sb, scalar1=bc_ps[:, 0:1], scalar2=None,
                op0=ALU.subtract)
            t_ch = stats.tile([P, KC], F32, tag="t_ch")
            nc.vector.tensor_mul(t_ch, u, s_ch)
            nc.vector.tensor_add(t_ch, t_ch, beta_sb)
            st_scale[b] = (s_ch, t_ch)

        def emit_By_setup(b):
            x_blks = {}
            for ns in range(NS):
                xb = xpool.tile([P, KC, 512], F16, tag="xblk",
                                name=f"x_{b}_{ns}")
                nc.sync.dma_start(
                    out=xb, in_=x_d[b, ns].rearrange("p (k n) -> p k n", k=KC))
                x_blks[ns] = xb
            st_by[b] = (x_blks, {})

        def emit_By_blocks(b, blocks):
            """y = W_eff @ x with the groupnorm apply FUSED into the psum
            evacuation (s/t known up front); finished [P,1024] output tiles
            are DMA'd to HBM immediately."""
            w2 = st_w2[b]
            s_ch, t_ch = st_scale[b]
            x_blks, ybufs = st_by[b]
            for ns in blocks:
                pair, half = ns // 2, ns % 2
                x_blk = x_blks.pop(ns)
                for m in range(KC):
                    if half == 0:
                        yb = ybuf.tile([P, 1024], F16, tag="yb",
                                       name=f"yb_{b}_{m}_{pair}")
                        ybufs[(m, pair)] = yb
                    else:
                        yb = ybufs[(m, pair)]
                    ps = psmm.tile([P, 512], F32, tag="psmm")
                    for k in range(KC):
                        nc.tensor.matmul(
                            ps,
                            lhsT=w2[:, k, m * P:(m + 1) * P],
                            rhs=x_blk[:, k, :],
                            start=(k == 0), stop=(k == KC - 1))
                    ysl = yb[:, half * 512:(half + 1) * 512]
                    if m % 2 == 1:
                        nc.vector.tensor_scalar(
                            out=ysl, in0=ps,
                            scalar1=s_ch[:, m:m + 1], scalar2=t_ch[:, m:m + 1],
                            op0=ALU.mult, op1=ALU.add)
                    else:
                        nc.scalar.activation(
                            out=ysl, in_=ps, func=ACTF.Identity,
                            bias=t_ch[:, m:m + 1], scale=s_ch[:, m:m + 1])
                    if half == 1:
                        nc.sync.dma_start(
                            out=out_d[b, m * P:(m + 1) * P,
                                      pair * 1024:(pair + 1) * 1024],
                            in_=yb)

        # ---- emission schedule (PE queue order is emission order) ----
        emit_G(0, prefetched=XLOOK, hook={
            NT - 1 - XLOOK: lambda: W.update(wk=load_w(wk_d),
                                             wq=load_w(wq_d))})
        W.update(wo=load_w(wo_d), wv=load_w(wv_d))
        emit_T(0)
        emit_scores(0)
        emit_softmax(0)       # DVE, overlaps G1a on PE
        emit_G(1, prefetched=XLOOK, t_range=range(4))
        emit_R(0)
        emit_W2(0)
        emit_M(0)
        emit_stats_a(0)       # DVE reads M psum, overlaps G1b on PE
        emit_ws(0)
        emit_stats_sv(0)
        emit_G(1, prefetched=XLOOK, t_range=range(4, NT))
        emit_stats_mm(0)
        emit_stats_b(0)       # DVE, fully covered by G1b
        emit_T(1)
        emit_stats_bc(0)
        emit_stats_c(0)
        emit_scores(1)
        emit_softmax(1)       # DVE, overlaps By0 on PE
        emit_By_setup(0)
        emit_By_blocks(0, range(6))
        emit_R(1)
        emit_W2(1)
        emit_M(1)
        emit_stats_a(1)       # DVE, overlaps By0 block 6 on PE
        emit_By_blocks(0, range(6, 7))
        emit_ws(1)
        emit_stats_sv(1)
        emit_stats_mm(1)
        emit_stats_b(1)       # DVE, overlaps By0 block 7 on PE
        emit_By_blocks(0, range(7, NS))
        emit_stats_bc(1)
        emit_stats_c(1)
        emit_By_setup(1)
        emit_By_blocks(1, range(NS))

    nc.finalize()
    return nc


_NC_CACHE = {}


def _get_nc():
    if "nc" not in _NC_CACHE:
        _NC_CACHE["nc"] = build_nc()
    return _NC_CACHE["nc"]


def _prep_w(w):
    # [C_in, C_out] -> [128, KC, C_out] fp16 with c_in = k*128 + p
    return np.ascontiguousarray(
        w.reshape(KC, P, C).transpose(1, 0, 2).astype(np.float16))


def _prep_vec(v):
    # [C] -> [128, KC] with c = k*128 + p
    return np.ascontiguousarray(v.reshape(KC, P).T)


def _prep_x(x):
    # [B, C, N] -> [B, NB, P, KC*512] fp16: block j, partition p, (k, n)
    nb = x.shape[0]
    xr = x.reshape(nb, KC, P, NB, 512)
    return np.ascontiguousarray(
        xr.transpose(0, 3, 2, 1, 4).astype(np.float16)).reshape(
        nb, NB, P, KC * 512)


def _prep_xt(x):
    # [B, C, N] -> [B, NT, P, 2*512] fp16: xt[b,t,p,j*512+c] = x[b,c,(2t+j)*128+p]
    nb = x.shape[0]
    xr = x.reshape(nb, C, NT, 2, P)           # [b, c, t, j, p]
    return np.ascontiguousarray(
        xr.transpose(0, 2, 4, 3, 1).astype(np.float16)).reshape(
        nb, NT, P, 1024)


def _make_in_maps(x, w_qkv, w_out, b_out, gamma, beta):
    x = np.asarray(x, dtype=np.float32).reshape(B, C, N)
    xr = _prep_x(x)
    xtr = _prep_xt(x)
    # xs[p, b, k] = sum_n x[b, k*128+p, n]
    xs = x.sum(axis=2).reshape(B, KC, P).transpose(2, 0, 1)
    xs = np.ascontiguousarray(xs.astype(np.float16))
    w_qkv = np.asarray(w_qkv, dtype=np.float32)
    wq = _prep_w(np.ascontiguousarray(w_qkv[0:C].T))
    wk = _prep_w(np.ascontiguousarray(w_qkv[C:2 * C].T))
    wv = _prep_w(np.ascontiguousarray(w_qkv[2 * C:3 * C]))
    wo = _prep_w(np.ascontiguousarray(np.asarray(w_out, dtype=np.float32).T))
    b_out = np.asarray(b_out, dtype=np.float32)
    bvec = _prep_vec(b_out)
    gam = _prep_vec(np.asarray(gamma, dtype=np.float32))
    bet = _prep_vec(np.asarray(beta, dtype=np.float32))
    bconst = np.array([[b_out.sum() / C, (b_out * b_out).sum() / C + EPS]],
                      dtype=np.float32)
    return [
        dict(x=np.ascontiguousarray(xr[c * PB:(c + 1) * PB]),
             xt=np.ascontiguousarray(xtr[c * PB:(c + 1) * PB]),
             xs=np.ascontiguousarray(xs[:, c * PB:(c + 1) * PB]),
             wq=wq, wk=wk, wv=wv, wo=wo,
             bvec=bvec, gamma=gam, beta=bet, bconst=bconst)
        for c in range(NCORES)
    ]


def _run(inputs, trace=False, trace_kwargs=None):
    nc = _get_nc()
    in_maps = _make_in_maps(**inputs)
    res = run_bass_kernel_spmd(nc, in_maps, core_ids=list(range(NCORES)),
                               trace=trace, **(trace_kwargs or {}))
    out = np.concatenate([res.results[c]["out"].astype(np.float32)
                          for c in range(NCORES)], axis=0)
    return out.reshape(B, C, HW_SIDE, HW_SIDE), res


def kernel(x, w_qkv, w_out, b_out, gamma, beta):
    inputs = dict(x=x, w_qkv=w_qkv, w_out=w_out, b_out=b_out,
                  gamma=gamma, beta=beta)
    try:
        out, _ = _run(inputs)
    except Exception:
        # transient device errors (e.g. NRT_EXEC_UNIT_UNRECOVERABLE) have
        # been observed once across many runs; one retry recovers.
        out, _ = _run(inputs)
    return out


# revision 44
# speedup vs baseline: 1.1490x; 1.0685x over previous
"""Trainium2 Bass kernel for nn_MultiHeadAttention_63814624084186.

Reference computation (per batch sample b, fully independent across b):
  x: [512, 4096]  (C channels x N=64*64 pixels)
  qkv = w_qkv @ x            -> q,k,v each [512, 4096] (8 heads x 64 dims)
  scores = (q_h @ k_h^T)/8   -> [64, 64] per head   (channel-attention)
  attn = softmax(scores, -1)
  out_h = attn_h @ v_h       -> [64, 4096]
  y = w_out @ out + b_out    -> [512, 4096]
  y = groupnorm(y over all C,N) * gamma + beta

Key algebra (this version): attention is over the CHANNEL dim, so
  scores_h = q_h k_h^T = (w_q G w_k^T)_h   with  G = x x^T  [512,512]
  y = w_out bd(A) w_v x = W_eff x          with  W_eff folded on-chip
q, k, v are never materialized.  Per-batch PE work drops from ~4.5e9
MACs (qkv + v + out-proj) to ~2.4e9 (G + y GEMM + small folds).

GroupNorm stats are computed ANALYTICALLY before the y GEMM runs:
  sum(y)   = 1^T W_eff xs        (xs = sum_n x, host-prepared)
  sum(y^2) = tr((W_eff^T W_eff) G) = <M, G>   (G already on chip)
so the per-channel scale/offset are known up front, the normalization is
fused into the y-psum evacuation, and each output tile is DMA'd to HBM
as soon as it is produced: no bn_stats, no apply pass, no output drain.

Sharding: pure data-parallel over batch: 16 samples / 8 cores = 2 per core.

Pipeline (PE queue order; b0/b1 are the two per-core batches):
  G0 T0 sc0 | G1 | R0 W20 M0 | T1 sc1 (stat mms b0) | By0[0:6] |
  R1 W21 M1 | By0[6:8] (stat mms b1) | By1
softmax/stat chains run on DVE under the covering PE phases.
"""

import numpy as np
from contextlib import ExitStack

import concourse.bass as bass
import concourse.tile as tile
from concourse import bacc, mybir
from concourse.bass_utils import run_bass_kernel_spmd
from concourse.masks import make_identity

F32 = mybir.dt.float32
F16 = mybir.dt.float16
I32 = mybir.dt.int32
AX = mybir.AxisListType
ALU = mybir.AluOpType
ACTF = mybir.ActivationFunctionType

B = 16          # global batch
C = 512         # channels
N = 4096        # pixels (64*64)
HW_SIDE = 64
NCORES = 8
PB = B // NCORES  # batches per core
P = 128
KC = C // P     # 4 channel chunks
NB = 8          # n blocks of 512 (y GEMM)
NT = 16         # xT tiles of 2 n-chunks each (G GEMM)
NS = N // 512   # 8 pixel chunks of 512
NHP = 4         # head pairs
XLOOK = 8       # xT DMA lookahead tiles
EPS = 1e-5
CN = C * N


def build_nc():
    nc = bacc.Bacc("TRN2", target_bir_lowering=False, debug=False,
                   num_devices=NCORES)

    # xT[b, t, p, j*512+c] = x[b, c, (2t+j)*128 + p]
    xt_d = nc.declare_dram_parameter("xt", [PB, NT, P, 1024], F16, isOutput=False)
    # x[b, nb, p, k*512+n] = x[b, k*128+p, nb*512+n]
    x_d = nc.declare_dram_parameter("x", [PB, NB, P, KC * 512], F16, isOutput=False)
    wq_d = nc.declare_dram_parameter("wq", [P, KC, C], F16, isOutput=False)   # w_q^T
    wk_d = nc.declare_dram_parameter("wk", [P, KC, C], F16, isOutput=False)   # w_k^T
    wv_d = nc.declare_dram_parameter("wv", [P, KC, C], F16, isOutput=False)   # w_v
    wo_d = nc.declare_dram_parameter("wo", [P, KC, C], F16, isOutput=False)   # w_out^T
    xs_d = nc.declare_dram_parameter("xs", [P, PB, KC], F16, isOutput=False)
    bias_d = nc.declare_dram_parameter("bvec", [P, KC], F32, isOutput=False)
    gamma_d = nc.declare_dram_parameter("gamma", [P, KC], F32, isOutput=False)
    beta_d = nc.declare_dram_parameter("beta", [P, KC], F32, isOutput=False)
    bconst_d = nc.declare_dram_parameter("bconst", [1, 2], F32, isOutput=False)
    out_d = nc.declare_dram_parameter("out", [PB, C, N], F16, isOutput=True)

    with tile.TileContext(nc) as tc, ExitStack() as ctx:
        consts = ctx.enter_context(tc.tile_pool(name="consts", bufs=1))
        xtpool = ctx.enter_context(tc.tile_pool(name="xtpool", bufs=XLOOK + 2))
        xpool = ctx.enter_context(tc.tile_pool(name="xpool", bufs=10))
        gpool = ctx.enter_context(tc.tile_pool(name="gpool", bufs=2))
        tpool = ctx.enter_context(tc.tile_pool(name="tpool", bufs=2))
        rpool = ctx.enter_context(tc.tile_pool(name="rpool", bufs=2))
        w2pool = ctx.enter_context(tc.tile_pool(name="w2pool", bufs=2))
        w2tpool = ctx.enter_context(tc.tile_pool(name="w2tpool", bufs=2))
        mpool = ctx.enter_context(tc.tile_pool(name="mpool", bufs=2))
        ybuf = ctx.enter_context(tc.tile_pool(name="ybuf", bufs=10))
        attn = ctx.enter_context(tc.tile_pool(name="attn", bufs=8))
        attnt = ctx.enter_context(tc.tile_pool(name="attnt", bufs=4))
        stats = ctx.enter_context(tc.tile_pool(name="stats", bufs=4))
        # psg serves G (4 full banks), the scores tiles and the w2-transpose
        # staging: a matmul start=True resets the target bank's whole
        # per-partition row, so concurrent accumulators need separate banks
        # (partition packing 0:64/64:128 within a bank is safe, free-offset
        # packing is NOT).
        psg = ctx.enter_context(tc.tile_pool(name="psg", bufs=4, space="PSUM"))
        psmm = ctx.enter_context(tc.tile_pool(name="psmm", bufs=4, space="PSUM"))

        def load_w(dram):
            t = consts.tile([P, KC, C], F16, tag=f"w_{dram.name}")
            nc.sync.dma_start(out=t, in_=dram[:, :, :])
            return t

        xt_tiles = {}

        def fetch_xt(b, t):
            xt = xtpool.tile([P, 2, 512], F16, tag="xt", name=f"xt_{b}_{t}")
            nc.sync.dma_start(
                out=xt, in_=xt_d[b, t].rearrange("p (j c) -> p j c", j=2))
            xt_tiles[(b, t)] = xt

        # xT tiles first: weight loads are deferred until G0's xT stream is
        # fully issued (weights are only needed from T0 onward), so the
        # first G matmuls are never DMA-supply-gated.
        for t in range(XLOOK):
            fetch_xt(0, t)
        W = {}

        xs_sb = consts.tile([P, PB, KC], F16, tag="xs")
        nc.gpsimd.dma_start(out=xs_sb, in_=xs_d[:, :, :])
        bias_sb = consts.tile([P, KC], F32, tag="bias")
        nc.gpsimd.dma_start(out=bias_sb, in_=bias_d[:, :])
        gamma_sb = consts.tile([P, KC], F32, tag="gamma")
        nc.gpsimd.dma_start(out=gamma_sb, in_=gamma_d[:, :])
        beta_sb = consts.tile([P, KC], F32, tag="beta")
        nc.gpsimd.dma_start(out=beta_sb, in_=beta_d[:, :])
        bconst_sb = consts.tile([1, 2], F32, tag="bconst")
        nc.gpsimd.dma_start(out=bconst_sb, in_=bconst_d[:, :])

        ident_sb = consts.tile([P, P], F16, tag="ident")
        make_identity(nc, ident_sb)
        eps_sb = consts.tile([1, 1], F32, tag="eps")
        nc.vector.memset(eps_sb, EPS)
        # pre-warm the exp activation table (softmax) so no ACT table load
        # lands mid-kernel; the stats chain's rsqrt runs DVE-only.
        warm_sb = consts.tile([1, 1], F32, tag="warm")
        nc.scalar.activation(out=warm_sb, in_=eps_sb, func=ACTF.Exp,
                             bias=0.0, scale=0.0)
        magic_sb = consts.tile([1, 1], I32, tag="magic")
        nc.vector.memset(magic_sb, 0x5f3759df)
        c15_sb = consts.tile([1, 1], F32, tag="c15")
        nc.vector.memset(c15_sb, 1.5)
        ones_col = consts.tile([P, 1], F32, tag="ones_col")
        nc.vector.memset(ones_col, 1.0)
        ones_row = consts.tile([1, P], F32, tag="ones_row")
        nc.vector.memset(ones_row, 1.0)

        # per-batch state carried between emission stages
        st_g = {}    # G in SBUF (f16) [P, KC, C]
        st_t = {}    # T = G @ wk^T   [P, KC, C]
        st_sc = {}   # scores psum tiles
        st_at = {}   # block-diag attn tiles
        st_r = {}    # R = bd(A)^T @ wo^T
        st_w2 = {}   # W_effT = wv^T-contract @ R
        st_ts = {}   # analytic-stats intermediates
        st_scale = {}
        st_by = {}

        st_gps = {}

        def emit_G(b, prefetched, hook=None, t_range=None):
            """G = x x^T, t-outer: each xT tile is consumed then retired.
            Only the upper block-triangle is computed (rhs = cols >= m*128);
            the 6 lower [128,128] blocks are PE-transposed from the upper
            copies.  All 4 chunk accumulators live in 4 psum banks.
            t_range allows splitting the accumulation into several emission
            segments (psum groups stay open in between)."""
            if t_range is None:
                t_range = range(NT)
            if t_range[0] == 0:
                g_sb = gpool.tile([P, KC, C], F16, tag="g", name=f"g_{b}")
                st_g[b] = g_sb
                st_gps[b] = [psg.tile([P, C - m * P], F32, tag="psg",
                                      name=f"g_{b}_{m}") for m in range(KC)]
            g_sb = st_g[b]
            ps = st_gps[b]
            for t in t_range:
                tf = t + prefetched
                if tf < NT:
                    fetch_xt(b, tf)
                elif b + 1 < PB and tf - NT < NT:
                    fetch_xt(b + 1, tf - NT)
                if hook and t in hook:
                    hook[t]()
                xt = xt_tiles.pop((b, t))
                for j in range(2):
                    for m in range(KC):
                        nc.tensor.matmul(
                            ps[m],
                            lhsT=xt[:, j, m * P:(m + 1) * P],
                            rhs=xt[:, j, m * P:],
                            start=(t == 0 and j == 0),
                            stop=(t == NT - 1 and j == 1),
                            skip_group_check=True)
            if t_range[-1] != NT - 1:
                return
            for m in range(KC):
                if m % 2 == 0:
                    nc.scalar.copy(out=g_sb[:, m, m * P:], in_=ps[m])
                else:
                    nc.vector.tensor_copy(out=g_sb[:, m, m * P:], in_=ps[m])
            # lower blocks (m, mp<m) = transpose(upper block (mp, m)),
            # ordered so T chunk 0's operands are ready first
            for m, mp in ((1, 0), (2, 0), (3, 0), (2, 1), (3, 1), (3, 2)):
                pst = psmm.tile([P, P], F16, tag="psmm")
                nc.tensor.transpose(
                    pst, g_sb[:, mp, m * P:(m + 1) * P], ident_sb)
                if (m + mp) % 2 == 0:
                    nc.vector.tensor_copy(
                        out=g_sb[:, m, mp * P:(mp + 1) * P], in_=pst)
                else:
                    nc.scalar.copy(
                        out=g_sb[:, m, mp * P:(mp + 1) * P], in_=pst)

        def emit_T(b):
            """T = G @ wk^T  [c, e], m-outer single-bank accumulation."""
            g_sb = st_g[b]
            t_sb = tpool.tile([P, KC, C], F16, tag="t", name=f"t_{b}")
            st_t[b] = t_sb
            for m in range(KC):
                ps = psmm.tile([P, C], F32, tag="psmm")
                for mp in range(KC):
                    nc.tensor.matmul(
                        ps,
                        lhsT=g_sb[:, mp, m * P:(m + 1) * P],
                        rhs=W['wk'][:, mp, :],
                        start=(mp == 0), stop=(mp == KC - 1))
                nc.vector.tensor_copy(out=t_sb[:, m, :], in_=ps)

        def emit_scores(b):
            """scores_h = (w_q T)_h, two heads packed per psum tile; k-outer
            so the first matmuls only need T chunk 0."""
            t_sb = st_t[b]
            sc_ps = [psg.tile([P, 64], F32, tag="psg", name=f"sc_{b}_{hp}")
                     for hp in range(NHP)]
            st_sc[b] = sc_ps
            for k in range(KC):
                for hp in range(NHP):
                    hA, hB = 2 * hp, 2 * hp + 1
                    clA = slice(hA * 64, hA * 64 + 64)
                    clB = slice(hB * 64, hB * 64 + 64)
                    nc.tensor.matmul(
                        sc_ps[hp][0:64, :],
                        lhsT=W['wq'][:, k, clA], rhs=t_sb[:, k, clA],
                        start=(k == 0), stop=(k == KC - 1),
                        skip_group_check=True)
                    nc.tensor.matmul(
                        sc_ps[hp][64:P, :],
                        lhsT=W['wq'][:, k, clB], rhs=t_sb[:, k, clB],
                        start=(k == 0), stop=(k == KC - 1),
                        skip_group_check=True)

        def emit_softmax(b):
            """softmax over scores (all head pairs batched) -> blockdiag tiles."""
            sc_ps = st_sc[b]
            a_all = attn.tile([P, NHP, 64], F32, tag="a_all")
            for hp in range(NHP):
                nc.vector.tensor_copy(out=a_all[:, hp, :], in_=sc_ps[hp])
            mx = attn.tile([P, NHP, 1], F32, tag="mx4")
            nc.vector.reduce_max(out=mx, in_=a_all, axis=AX.X)
            d_all = attn.tile([P, NHP, 64], F32, tag="d_all")
            nc.vector.tensor_tensor(d_all, a_all,
                                    mx.to_broadcast([P, NHP, 64]), ALU.subtract)
            e_all = attn.tile([P, NHP, 64], F32, tag="e_all")
            nc.scalar.activation(out=e_all, in_=d_all, func=ACTF.Exp,
                                 bias=0.0, scale=0.125)
            sm = attn.tile([P, NHP, 1], F32, tag="sm4")
            nc.vector.reduce_sum(out=sm, in_=e_all, axis=AX.X)
            rs = attn.tile([P, NHP, 1], F32, tag="rs4")
            nc.vector.reciprocal(out=rs, in_=sm)
            a_mm = attn.tile([P, NHP, 64], F16, tag="amm4")
            nc.vector.tensor_tensor(a_mm, e_all,
                                    rs.to_broadcast([P, NHP, 64]), ALU.mult)
            bd_tiles = []
            for hp in range(NHP):
                at = attnt.tile([P, P], F16, tag="attnT", name=f"at_{b}_{hp}")
                nc.gpsimd.memset(at, 0.0)
                nc.vector.tensor_copy(out=at[0:64, 0:64], in_=a_mm[0:64, hp, :])
                nc.vector.tensor_copy(out=at[64:P, 64:P], in_=a_mm[64:P, hp, :])
                bd_tiles.append(at)
            st_at[b] = bd_tiles

        def emit_R(b):
            """R[e, o] = sum_d bd(A)[d, e] wo^T[d, o]."""
            bd_tiles = st_at[b]
            r_sb = rpool.tile([P, KC, C], F16, tag="r", name=f"r_{b}")
            st_r[b] = r_sb
            for hp in range(NHP):
                ps = psmm.tile([P, C], F32, tag="psmm")
                nc.tensor.matmul(ps, lhsT=bd_tiles[hp], rhs=W['wo'][:, hp, :],
                                 start=True, stop=True)
                if hp % 2 == 0:
                    nc.scalar.copy(out=r_sb[:, hp, :], in_=ps)
                else:
                    nc.vector.tensor_copy(out=r_sb[:, hp, :], in_=ps)

        def emit_W2(b):
            """W_effT[c, o] = sum_e wv[e, c] R[e, o], m-outer."""
            r_sb = st_r[b]
            w2 = w2pool.tile([P, KC, C], F16, tag="w2", name=f"w2_{b}")
            st_w2[b] = w2
            for m in range(KC):
                ps = psmm.tile([P, C], F32, tag="psmm")
                for ki in range(KC):
                    nc.tensor.matmul(
                        ps,
                        lhsT=W['wv'][:, ki, m * P:(m + 1) * P],
                        rhs=r_sb[:, ki, :],
                        start=(ki == 0), stop=(ki == KC - 1))
                if m % 2 == 0:
                    nc.scalar.copy(out=w2[:, m, :], in_=ps)
                else:
                    nc.vector.tensor_copy(out=w2[:, m, :], in_=ps)

        def emit_M(b, mm_pool=None):
            """M = W_eff^T W_eff (upper block-triangle) + ws = W_eff @ xs.
            w2 is transposed on the PE (mo-outer so the M accumulation can
            chase the transpose copies), then M accumulates like G."""
            w2 = st_w2[b]
            w2t = w2tpool.tile([P, KC, C], F16, tag="w2t", name=f"w2t_{b}")
            # transposes go through the psmm rotation (psg may hold an open
            # G accumulation while this runs); all 16 land before the M mms
            for mo in range(KC):
                for k in range(KC):
                    pst = psmm.tile([P, P], F16, tag="psmm")
                    nc.tensor.transpose(
                        pst, w2[:, k, mo * P:(mo + 1) * P], ident_sb)
                    if k % 2 == 0:
                        nc.scalar.copy(
                            out=w2t[:, mo, k * P:(k + 1) * P], in_=pst)
                    else:
                        nc.vector.tensor_copy(
                            out=w2t[:, mo, k * P:(k + 1) * P], in_=pst)
            pool = mm_pool or psmm
            tagp = "psg" if pool is psg else "psmm"
            ps = [pool.tile([P, C - m * P], F32, tag=tagp, name=f"M_{b}_{m}")
                  for m in range(KC)]
            for mo in range(KC):
                for m in range(KC):
                    nc.tensor.matmul(
                        ps[m],
                        lhsT=w2t[:, mo, m * P:(m + 1) * P],
                        rhs=w2t[:, mo, m * P:],
                        start=(mo == 0), stop=(mo == KC - 1),
                        skip_group_check=True)
            st_ts[b] = {"mps": ps}

        def emit_ws(b):
            """ws[o] = sum_c W_eff[o, c] xs[c] (PE, tiny)."""
            w2 = st_w2[b]
            ws_sb = stats.tile([P, KC], F32, tag="ws")
            for m in range(KC):
                wps = psmm.tile([P, 1], F32, tag="psmm")
                for k in range(KC):
                    nc.tensor.matmul(
                        wps,
                        lhsT=w2[:, k, m * P:(m + 1) * P],
                        rhs=xs_sb[:, b, k:k + 1],
                        start=(k == 0), stop=(k == KC - 1))
                nc.vector.tensor_copy(out=ws_sb[:, m:m + 1], in_=wps)
            st_ts[b]["ws"] = ws_sb

        def emit_stats_a(b):
            """DVE: tr(M G) = sum(2*upper - diag) read straight from the
            M psum banks (no SBUF staging) -> trp partials."""
            mps = st_ts[b]["mps"]
            g_sb = st_g[b]
            trp = stats.tile([P, 8], F32, tag="trp")
            scr = stats.tile([P, C], F32, tag="scr")
            for m in range(KC):
                w = C - m * P
                nc.vector.tensor_tensor(
                    scr[:, :w], mps[m], g_sb[:, m, m * P:], ALU.mult)
                nc.vector.reduce_sum(out=trp[:, m:m + 1], in_=scr[:, :w],
                                     axis=AX.X)
                nc.vector.reduce_sum(out=trp[:, KC + m:KC + m + 1],
                                     in_=scr[:, 0:P], axis=AX.X)
            st_ts[b]["trp"] = trp

        def emit_stats_sv(b):
            """sv = [2*sum(upper) - sum(diag), sum(ws), sum(ws*b)]."""
            trp = st_ts[b]["trp"]
            ws = st_ts[b]["ws"]
            sv = stats.tile([P, 3], F32, tag="sv")
            dd = stats.tile([P, 2], F32, tag="dd")
            nc.vector.reduce_sum(out=dd[:, 0:1], in_=trp[:, 0:KC], axis=AX.X)
            nc.vector.reduce_sum(out=dd[:, 1:2], in_=trp[:, KC:2 * KC],
                                 axis=AX.X)
            nc.vector.tensor_scalar(
                out=sv[:, 0:1], in0=dd[:, 0:1], scalar1=2.0, scalar2=None,
                op0=ALU.mult)
            nc.vector.tensor_sub(sv[:, 0:1], sv[:, 0:1], dd[:, 1:2])
            u = stats.tile([P, KC], F32, tag="u")
            nc.vector.tensor_mul(u, ws, bias_sb)
            nc.vector.reduce_sum(out=sv[:, 1:2], in_=ws, axis=AX.X)
            nc.vector.reduce_sum(out=sv[:, 2:3], in_=u, axis=AX.X)
            # sv[0] += 2*S2_partial so p3[0] = TR + 2*S2 in one reduce
            nc.vector.tensor_scalar(
                out=sv[:, 2:3], in0=sv[:, 2:3], scalar1=2.0, scalar2=None,
                op0=ALU.mult)
            nc.vector.tensor_add(sv[:, 0:1], sv[:, 0:1], sv[:, 2:3])
            st_ts[b]["sv"] = sv

        def emit_stats_mm(b):
            """cross-partition reduce of [tr, S1, S2] (PE)."""
            sv = st_ts[b]["sv"]
            p3 = psmm.tile([1, 3], F32, tag="psmm", name=f"p3_{b}")
            nc.tensor.matmul(p3, lhsT=ones_col, rhs=sv, start=True, stop=True)
            st_ts[b]["p3"] = p3

        def emit_stats_b(b):
            """scalar combine: mu, var, rstd (DVE-only quake rsqrt)."""
            p3 = st_ts[b]["p3"]
            sc2 = stats.tile([1, 2], F32, tag="sc2")
            # mu = S1/CN + B1/C (fused)
            nc.vector.tensor_scalar(
                out=sc2[:, 0:1], in0=p3[:, 1:2], scalar1=1.0 / CN,
                scalar2=bconst_sb[:, 0:1], op0=ALU.mult, op1=ALU.add)
            # var+eps = (TR + 2*S2)/CN + (B2/C + eps) - mu^2
            vb = stats.tile([1, 1], F32, tag="vb")
            nc.vector.tensor_scalar(
                out=vb, in0=p3[:, 0:1], scalar1=1.0 / CN,
                scalar2=bconst_sb[:, 1:2], op0=ALU.mult, op1=ALU.add)
            m2 = stats.tile([1, 1], F32, tag="m2")
            nc.vector.tensor_mul(m2, sc2[:, 0:1], sc2[:, 0:1])
            nc.vector.tensor_sub(vb, vb, m2)
            # rstd = 1/sqrt(vb): quake bit trick + 1 Newton step (DVE only)
            hv = stats.tile([1, 1], F32, tag="hv")
            nc.vector.tensor_scalar_mul(hv, vb, 0.5)
            r = stats.tile([1, 1], F32, tag="rq")
            nc.vector.tensor_scalar(
                out=r.bitcast(I32), in0=vb.bitcast(I32),
                scalar1=1, scalar2=None, op0=ALU.logical_shift_right)
            nc.vector.tensor_tensor(r.bitcast(I32), magic_sb,
                                    r.bitcast(I32), ALU.subtract)
            t1 = stats.tile([1, 1], F32, tag="t1")
            for _ in range(2):
                nc.vector.tensor_mul(t1, r, r)
                nc.vector.tensor_mul(t1, hv, t1)
                nc.vector.tensor_sub(t1, c15_sb, t1)
                nc.vector.tensor_mul(r, r, t1)
            nc.vector.tensor_copy(out=sc2[:, 1:2], in_=r)
            st_ts[b]["sc2"] = sc2

        def emit_stats_bc(b):
            """broadcast mu/rstd to all partitions (PE)."""
            sc2 = st_ts[b]["sc2"]
            bc_ps = psmm.tile([P, 2], F32, tag="psmm", name=f"bc_{b}")
            nc.tensor.matmul(bc_ps, lhsT=ones_row, rhs=sc2,
                             start=True, stop=True)
            st_ts[b]["bc"] = bc_ps

        def emit_stats_c(b):
            """per-channel scale/offset: s = gamma*rstd,
            t = beta + (bias - mu)*s."""
            bc_ps = st_ts[b]["bc"]
            s_ch = stats.tile([P, KC], F32, tag="s_ch")
            nc.vector.tensor_scalar_mul(s_ch, gamma_sb, bc_ps[:, 1:2])
            u = stats.tile([P, KC], F32, tag="tb")
            nc.vector.tensor_scalar(
                out=u, in0=bias_sb, scalar1=bc_ps[:, 0:1], scalar2=None,
                op0=ALU.subtract)
            t_ch = stats.tile([P, KC], F32, tag="t_ch")
            nc.vector.tensor_mul(t_ch, u, s_ch)
            nc.vector.tensor_add(t_ch, t_ch, beta_sb)
            st_scale[b] = (s_ch, t_ch)

        def emit_By_setup(b):
            x_blks = {}
            for ns in range(NS):
                xb = xpool.tile([P, KC, 512], F16, tag="xblk",
                                name=f"x_{b}_{ns}")
                nc.sync.dma_start(
                    out=xb, in_=x_d[b, ns].rearrange("p (k n) -> p k n", k=KC))
                x_blks[ns] = xb
            st_by[b] = (x_blks, {})

        def emit_By_blocks(b, blocks):
            """y = W_eff @ x with the groupnorm apply FUSED into the psum
            evacuation (s/t known up front); finished [P,1024] output tiles
            are DMA'd to HBM immediately."""
            w2 = st_w2[b]
            s_ch, t_ch = st_scale[b]
            x_blks, ybufs = st_by[b]
            for ns in blocks:
                pair, half = ns // 2, ns % 2
                x_blk = x_blks.pop(ns)
                for m in range(KC):
                    if half == 0:
                        yb = ybuf.tile([P, 1024], F16, tag="yb",
                                       name=f"yb_{b}_{m}_{pair}")
                        ybufs[(m, pair)] = yb
                    else:
                        yb = ybufs[(m, pair)]
                    ps = psmm.tile([P, 512], F32, tag="psmm")
                    for k in range(KC):
                        nc.tensor.matmul(
                            ps,
                            lhsT=w2[:, k, m * P:(m + 1) * P],
                            rhs=x_blk[:, k, :],
                            start=(k == 0), stop=(k == KC - 1))
                    ysl = yb[:, half * 512:(half + 1) * 512]
                    if m % 2 == 1:
                        nc.vector.tensor_scalar(
                            out=ysl, in0=ps,
                            scalar1=s_ch[:, m:m + 1], scalar2=t_ch[:, m:m + 1],
                            op0=ALU.mult, op1=ALU.add)
                    else:
                        nc.scalar.activation(
                            out=ysl, in_=ps, func=ACTF.Identity,
                            bias=t_ch[:, m:m + 1], scale=s_ch[:, m:m + 1])
                    if b == PB - 1 and pair == 3:
                        nc.sync.dma_start(
                            out=out_d[b, m * P:(m + 1) * P,
                                      ns * 512:(ns + 1) * 512],
                            in_=ysl)
                    elif half == 1:
                        nc.sync.dma_start(
                            out=out_d[b, m * P:(m + 1) * P,
                                      pair * 1024:(pair + 1) * 1024],
                            in_=yb)

        # ---- emission schedule (PE queue order is emission order) ----
        emit_G(0, prefetched=XLOOK, hook={
            NT - 1 - XLOOK: lambda: W.update(wk=load_w(wk_d),
                                             wq=load_w(wq_d))})
        W.update(wo=load_w(wo_d), wv=load_w(wv_d))
        emit_T(0)
        emit_scores(0)
        emit_softmax(0)       # DVE, overlaps G1a on PE
        emit_G(1, prefetched=XLOOK, t_range=range(4))
        emit_R(0)
        emit_W2(0)
        emit_M(0)
        emit_stats_a(0)       # DVE reads M psum, overlaps G1b on PE
        emit_ws(0)
        emit_stats_sv(0)
        emit_G(1, prefetched=XLOOK, t_range=range(4, NT))
        emit_stats_mm(0)
        emit_stats_b(0)       # DVE, fully covered by G1b
        emit_T(1)
        emit_stats_bc(0)
        emit_stats_c(0)
        emit_scores(1)
        emit_softmax(1)       # DVE, overlaps By0 on PE
        emit_By_setup(0)
        emit_By_blocks(0, range(5))
        emit_R(1)
        emit_W2(1)
        emit_M(1, mm_pool=psg)   # psg is free after sc1: keeps the psmm
        emit_stats_a(1)          # rotation clear for By0's tail blocks
        emit_By_blocks(0, range(5, 7))
        emit_ws(1)
        emit_stats_sv(1)
        emit_stats_mm(1)
        emit_stats_b(1)       # DVE, overlaps By0 block 7 on PE
        emit_By_blocks(0, range(7, NS))
        emit_stats_bc(1)
        emit_stats_c(1)
        emit_By_setup(1)
        emit_By_blocks(1, range(NS))

    nc.finalize()
    return nc


_NC_CACHE = {}


def _get_nc():
    if "nc" not in _NC_CACHE:
        _NC_CACHE["nc"] = build_nc()
    return _NC_CACHE["nc"]


def _prep_w(w):
    # [C_in, C_out] -> [128, KC, C_out] fp16 with c_in = k*128 + p
    return np.ascontiguousarray(
        w.reshape(KC, P, C).transpose(1, 0, 2).astype(np.float16))


def _prep_vec(v):
    # [C] -> [128, KC] with c = k*128 + p
    return np.ascontiguousarray(v.reshape(KC, P).T)


def _prep_x(x):
    # [B, C, N] -> [B, NB, P, KC*512] fp16: block j, partition p, (k, n)
    nb = x.shape[0]
    xr = x.reshape(nb, KC, P, NB, 512)
    return np.ascontiguousarray(
        xr.transpose(0, 3, 2, 1, 4).astype(np.float16)).reshape(
        nb, NB, P, KC * 512)


def _prep_xt(x):
    # [B, C, N] -> [B, NT, P, 2*512] fp16: xt[b,t,p,j*512+c] = x[b,c,(2t+j)*128+p]
    nb = x.shape[0]
    xr = x.reshape(nb, C, NT, 2, P)           # [b, c, t, j, p]
    return np.ascontiguousarray(
        xr.transpose(0, 2, 4, 3, 1).astype(np.float16)).reshape(
        nb, NT, P, 1024)


def _make_in_maps(x, w_qkv, w_out, b_out, gamma, beta):
    x = np.asarray(x, dtype=np.float32).reshape(B, C, N)
    xr = _prep_x(x)
    xtr = _prep_xt(x)
    # xs[p, b, k] = sum_n x[b, k*128+p, n]
    xs = x.sum(axis=2).reshape(B, KC, P).transpose(2, 0, 1)
    xs = np.ascontiguousarray(xs.astype(np.float16))
    w_qkv = np.asarray(w_qkv, dtype=np.float32)
    wq = _prep_w(np.ascontiguousarray(w_qkv[0:C].T))
    wk = _prep_w(np.ascontiguousarray(w_qkv[C:2 * C].T))
    wv = _prep_w(np.ascontiguousarray(w_qkv[2 * C:3 * C]))
    wo = _prep_w(np.ascontiguousarray(np.asarray(w_out, dtype=np.float32).T))
    b_out = np.asarray(b_out, dtype=np.float32)
    bvec = _prep_vec(b_out)
    gam = _prep_vec(np.asarray(gamma, dtype=np.float32))
    bet = _prep_vec(np.asarray(beta, dtype=np.float32))
    bconst = np.array([[b_out.sum() / C, (b_out * b_out).sum() / C + EPS]],
                      dtype=np.float32)
    return [
        dict(x=np.ascontiguousarray(xr[c * PB:(c + 1) * PB]),
             xt=np.ascontiguousarray(xtr[c * PB:(c + 1) * PB]),
             xs=np.ascontiguousarray(xs[:, c * PB:(c + 1) * PB]),
             wq=wq, wk=wk, wv=wv, wo=wo,
             bvec=bvec, gamma=gam, beta=bet, bconst=bconst)
        for c in range(NCORES)
    ]


def _run(inputs, trace=False, trace_kwargs=None):
    nc = _get_nc()
    in_maps = _make_in_maps(**inputs)
    res = run_bass_kernel_spmd(nc, in_maps, core_ids=list(range(NCORES)),
                               trace=trace, **(trace_kwargs or {}))
    out = np.concatenate([res.results[c]["out"].astype(np.float32)
                          for c in range(NCORES)], axis=0)
    return out.reshape(B, C, HW_SIDE, HW_SIDE), res


def kernel(x, w_qkv, w_out, b_out, gamma, beta):
    inputs = dict(x=x, w_qkv=w_qkv, w_out=w_out, b_out=b_out,
                  gamma=gamma, beta=beta)
    try:
        out, _ = _run(inputs)
    except Exception:
        # transient device errors (e.g. NRT_EXEC_UNIT_UNRECOVERABLE) have
        # been observed once across many runs; one retry recovers.
        out, _ = _run(inputs)
    return out
